# revision 1
# baseline (speedup 1.0000x reference)
"""Trainium2 Bass kernel: ViT-style global attention with decomposed
relative position bias (B=8, 32x32 tokens, dim 768, 12 heads, hd 64).

Sharding: data-parallel over batch B=8 -> one image per NeuronCore (8
cores), weights replicated, no collectives. TimelineSim: ~206.3 us/core.

Per-core dataflow (all on-chip ops partition-preserving; fp32r matmuls
at full PE rate, bf16 only where precision allows):
  1a  q/k features = Wqk @ xT (feature-major). q pre-scaled by hd^-0.5
      via host weight fold; per-partition bias applied during the
      PSUM->SBUF drain (ACT for q -> Q'ALL halves, DVE for k -> K'ALL
      halves + a bf16 staging copy STQB for the rel-pos matmuls).
      Odd heads are row-mirrored (rows 64:127) so every copy preserves
      partition indices.
  1b  RELH/RELW = rel-pos tables contracted against q: per 4 token-rows,
      one bf16 matmul with a block-diagonal stationary (prefetched,
      host-packed) computes even+odd heads of all 6 head-pairs at once;
      drained into Q'ALL = [qT | RELH | RELW] per head.
  1c  V token-major (xT stationary), V-bias via K=1 ones-row matmul,
      stored bf16 with a ones column and parity-dependent layout
      [V|1] / [0,1,0,V] -> softmax denominators and the AOD partition
      placement ride the PV matmul for free.
  2   attention per (head, kblock): ONE K=128 fp32r matmul produces
      scale*S^T + rel_h + rel_w in PSUM (the bias rides contraction rows
      64..127 against constant 0/1 indicator patterns stored in K'ALL).
      exp on ScalarE (PSUM->SBUF, bf16) -> P^T. PV matmul with the V''
      stationary accumulates (P@V)^T + the denominator row over kblocks.
      A K=1 ones-matmul broadcasts the denominator row; reciprocal
      (PSUM->SBUF) + multiply on DVE -> AOD feature-major. Denominators
      skip the max-subtraction (logits are bounded ~|2.5|) - safe in
      fp32. proj weights prefetched during attention.
  3   proj matmul over AOD (+proj_b per-partition on DVE) -> y^T
      feature-major, DMA'd out; the final transpose to token-major
      happens on the host during unsharding.
"""

import numpy as np

import concourse.bacc as bacc
import concourse.tile as tile
from concourse import mybir
from concourse import bass_utils

B, H, W, DIM = 8, 32, 32, 768
HEADS, HD = 12, 64
N = H * W  # 1024
NCORES = 8
SCALE = HD ** -0.5
F32 = mybir.dt.float32
F32R = mybir.dt.float32r
BF16 = mybir.dt.bfloat16
EXP = mybir.ActivationFunctionType.Exp
ADD = mybir.AluOpType.add

NC = DIM // 128      # 6 feature chunks
NKB = N // 128       # 8 key blocks
NQH = N // 512       # 2 query halves
VW = 65 + 128        # even (V|1) + odd (0,1,0,V) stationary widths

_CACHE = {}

import os
KNOB_KCOPY_ACT = os.environ.get("K_KCOPY", "0") == "1"   # k he-half on ACT
KNOB_STQB_ACT = os.environ.get("K_STQB", "0") == "1"     # STQB copy on ACT
KNOB_WCOPY_ACT = os.environ.get("K_WCOPY", "0") == "1"   # relw copies on ACT
KNOB_HCOPY_DVE = os.environ.get("K_HCOPY", "0") == "1"   # relh copies on DVE
KNOB_PT_BUFS = int(os.environ.get("K_PT", "7"))
KNOB_VCOPY_DVE = os.environ.get("K_VCOPY", "0") == "1"   # V copies on DVE


def build_nc():
    nc = bacc.Bacc("TRN2", target_bir_lowering=False, debug=False)

    xT = nc.dram_tensor("xT", (DIM, N), F32R, kind="ExternalInput").ap()
    wqkvT = nc.dram_tensor("wqkvT", (DIM, 3 * DIM), F32R, kind="ExternalInput").ap()
    qkvb = nc.dram_tensor("qkvb", (3 * DIM,), F32, kind="ExternalInput").ap()
    wprojT = nc.dram_tensor("wprojT", (DIM, DIM), F32R, kind="ExternalInput").ap()
    projb = nc.dram_tensor("projb", (DIM,), F32, kind="ExternalInput").ap()
    bdh = nc.dram_tensor("bdh", (128, H, 128), BF16, kind="ExternalInput").ap()
    bdw = nc.dram_tensor("bdw", (128, W, 128), BF16, kind="ExternalInput").ap()
    kconst = nc.dram_tensor("kconst", (64, N), F32R, kind="ExternalInput").ap()
    consd = nc.dram_tensor("consd", (128, 256), F32R, kind="ExternalInput").ap()
    vbrow = nc.dram_tensor("vbrow", (1, DIM), F32R, kind="ExternalInput").ap()
    y = nc.dram_tensor("y", (DIM, N), F32, kind="ExternalOutput").ap()

    qkvb2 = qkvb.rearrange("(c p one) -> c p one", p=128, one=1)   # [18][128,1]
    projb2 = projb.rearrange("(c p one) -> c p one", p=128, one=1)  # [6][128,1]
    bdh_r = bdh
    bdw_r = bdw
    IDN = mybir.ActivationFunctionType.Identity

    with tile.TileContext(nc) as tc:
        # PE p-state warm-up: the first real matmuls are DMA-gated for
        # ~12us while the PE would ramp at half rate for its first 3us of
        # busy time. Fill the idle window with throwaway matmuls so the
        # ramp completes before the first real matmul issues.
        nwarm = int(os.environ.get("K_WARM", "37"))
        if nwarm:
            with tc.tile_pool(name="warm", bufs=1) as warm_p, \
                 tc.tile_pool(name="warmps", bufs=1, space="PSUM") as wps_p:
                jnk = warm_p.tile([128, 512], BF16)
                nc.vector.memset(jnk, 0.5)
                jps = wps_p.tile([128, 512], F32)
                for _ in range(nwarm):
                    nc.tensor.matmul(jps, lhsT=jnk[:, 0:128], rhs=jnk,
                                     start=True, stop=True,
                                     skip_group_check=True)

        # ---- long-lived pools (bottom of SBUF stack) ----
        qall_p = tc.alloc_tile_pool(name="qall", bufs=1)
        kall_p = tc.alloc_tile_pool(name="kall", bufs=1)
        vall_p = tc.alloc_tile_pool(name="vall", bufs=1)
        cons_p = tc.alloc_tile_pool(name="cons", bufs=1)

        QALL = qall_p.tile([128, HEADS, N], F32R)
        KALL = kall_p.tile([128, HEADS, N], F32R)
        VALL = vall_p.tile([128, HEADS // 2, NKB, VW], BF16)
        CONS = cons_p.tile([128, 256], F32R)
        VBS = cons_p.tile([1, DIM], F32R)

        xt_p = tc.alloc_tile_pool(name="xtp", bufs=1)
        XT = xt_p.tile([128, NC, N], F32R, tag="xtslot")
        with tc.tile_pool(name="stage", bufs=1) as stage_p, \
             tc.tile_pool(name="wpool", bufs=1) as w_p, \
             tc.tile_pool(name="bias", bufs=4) as b_p, \
             tc.tile_pool(name="ps1", bufs=4, space="PSUM") as ps1_p:
            STQB = stage_p.tile([128, NC, N], BF16)
            # ---------- 1a: q & k features (feature-major) ----------
            def emit_qkv_group(g, first=False):
                wt = w_p.tile([128, NC, 384], F32R, tag="wt", bufs=2, name=f"wt{g}")
                for c in range(NC):
                    nc.sync.dma_start(
                        out=wt[:, c, :],
                        in_=wqkvT[c * 128:(c + 1) * 128, g * 384:(g + 1) * 384])
                for mi in range(3):
                    m = g * 3 + mi       # 0..11 (q: 0-5, k: 6-11)
                    bias_t = b_p.tile([128, 1], F32, tag="bias", name=f"b{m}")
                    nc.sync.dma_start(out=bias_t, in_=qkvb2[m])
                    for qh in range(NQH):
                        ps = ps1_p.tile([128, 512], F32, tag="ps1", bufs=int(os.environ.get("K_PS1","4")),
                                        name=f"ps1_{m}_{qh}")
                        for c in range(NC):
                            nc.tensor.matmul(
                                ps,
                                lhsT=wt[:, c, mi * 128:(mi + 1) * 128],
                                rhs=XT[:, c, qh * 512:(qh + 1) * 512],
                                start=(c == 0), stop=(c == NC - 1))
                        qsl = slice(qh * 512, (qh + 1) * 512)
                        he, ho = 2 * (m % 6), 2 * (m % 6) + 1
                        if m < 6:
                            nc.scalar.activation(QALL[0:64, he, qsl], ps[0:64],
                                                 IDN, bias=bias_t[0:64],
                                                 scale=1.0)
                            nc.scalar.activation(QALL[64:128, ho, qsl],
                                                 ps[64:128], IDN,
                                                 bias=bias_t[64:128], scale=1.0)
                            if KNOB_STQB_ACT:
                                nc.scalar.activation(STQB[:, m, qsl], ps, IDN,
                                                     bias=bias_t, scale=1.0)
                            else:
                                nc.vector.tensor_scalar(
                                    out=STQB[:, m, qsl], in0=ps,
                                    scalar1=bias_t, scalar2=None, op0=ADD)
                        else:
                            if KNOB_KCOPY_ACT:
                                nc.scalar.activation(
                                    KALL[0:64, he, qsl], ps[0:64], IDN,
                                    bias=bias_t[0:64], scale=1.0)
                            else:
                                nc.vector.tensor_scalar(
                                    out=KALL[0:64, he, qsl], in0=ps[0:64],
                                    scalar1=bias_t[0:64], scalar2=None, op0=ADD)
                            nc.vector.tensor_scalar(
                                out=KALL[64:128, ho, qsl], in0=ps[64:128],
                                scalar1=bias_t[64:128], scalar2=None, op0=ADD)

            for qh in range(NQH):
                for c in range(NC):
                    nc.sync.dma_start(
                        out=XT[:, c, qh * 512:(qh + 1) * 512],
                        in_=xT[c * 128:(c + 1) * 128, qh * 512:(qh + 1) * 512])
            emit_qkv_group(0)
            emit_qkv_group(1)
            bdts = []
            for i in range(16):
                src_r = bdh_r if i < 8 else bdw_r
                j0 = (i % 8) * 4
                bdt = w_p.tile([128, 4, 128], BF16, tag="bd", bufs=16,
                               name=f"bdt{i}")
                nc.sync.dma_start(out=bdt, in_=src_r[:, j0:j0 + 4, :])
                bdts.append(bdt)
            for g in range(2, 4):
                emit_qkv_group(g)

            # constants (after the critical 1a DMAs so they don't gate PE)
                nc.sync.dma_start(out=VBS, in_=vbrow)
            nc.sync.dma_start(out=CONS, in_=consd)
            # V'' layout: even head cols [V(64)|1]; odd [0(32)|1|0(31)|V(64)]
            nc.vector.memset(VALL[:, :, :, 64:65], 1.0)
            nc.vector.memset(VALL[:, :, :, 65:97], 0.0)
            nc.vector.memset(VALL[:, :, :, 97:98], 1.0)
            nc.vector.memset(VALL[:, :, :, 98:129], 0.0)

            # ---------- 1b: RELH/RELW ----------
            # out views with batched-h free-dim order (hb, pair, t)
            qvh = QALL.rearrange("p (pr hh) (hb t) -> p hb pr hh t", hh=2, t=W)
            qvw = QALL.rearrange("p (pr hh) (t wb) -> p wb pr hh t", hh=2, wb=W)
            stq4 = STQB.rearrange("p c (t ww) -> p c t ww", ww=W)
            RG = int(os.environ.get("K_RG", "4"))
            for i0 in range(0, H, RG):
                bdh_t = bdts[i0 // 4]
                ps_h = ps1_p.tile([128, RG, 256], F32, tag="ps2",
                                  bufs=int(os.environ.get("K_PS2", "2")),
                                  name=f"psh{i0}")
                for j in range(RG):
                    h = i0 + j
                    nc.tensor.matmul(
                        ps_h[:, j, 0:192].rearrange("p (c t) -> p c t", t=32),
                        lhsT=bdts[h // 4][:, h % 4, :],
                        rhs=STQB[:, :, h * 32:(h + 1) * 32],
                        start=True, stop=True, skip_group_check=True)
                _hcp = nc.vector.tensor_copy if KNOB_HCOPY_DVE else nc.scalar.copy
                _hcp(qvh[64:96, i0:i0 + RG, :, 0, :],
                     ps_h[64:96, :, 0:192].rearrange(
                         "p hb (c t) -> p hb c t", t=32))
                _hcp(qvh[0:32, i0:i0 + RG, :, 1, :],
                     ps_h[0:32, :, 0:192].rearrange(
                         "p hb (c t) -> p hb c t", t=32))
                ps_w = ps1_p.tile([128, RG, 256], F32, tag="ps2",
                                  bufs=int(os.environ.get("K_PS2", "2")),
                                  name=f"psw{i0}")
                for j in range(RG):
                    w = i0 + j
                    nc.tensor.matmul(
                        ps_w[:, j, 0:192].rearrange("p (c t) -> p c t", t=32),
                        lhsT=bdts[8 + w // 4][:, w % 4, :],
                        rhs=stq4[:, :, :, w],
                        start=True, stop=True, skip_group_check=True)
                _wcp = nc.scalar.copy if KNOB_WCOPY_ACT else nc.vector.tensor_copy
                _wcp(qvw[96:128, i0:i0 + RG, :, 0, :],
                     ps_w[96:128, :, 0:192].rearrange(
                         "p wb (c t) -> p wb c t", t=32))
                _wcp(qvw[32:64, i0:i0 + RG, :, 1, :],
                     ps_w[32:64, :, 0:192].rearrange(
                         "p wb (c t) -> p wb c t", t=32))

            # ---------- 1c: V token-major (vh=0: heads 0-5) ----------
            def emit_v_half(vh, wt, pool, tag, bufs):
                for tb in range(NKB):    # 8 token blocks
                    ps = pool.tile([128, 512], F32, tag=tag,
                                   bufs=int(os.environ.get("K_PS1", "4")),
                                   name=f"psv{vh}_{tb}")[:, 0:384]
                    for c in range(NC):
                        nc.tensor.matmul(
                            ps, lhsT=XT[:, c, tb * 128:(tb + 1) * 128],
                            rhs=wt[:, c, :],
                            start=(c == 0), stop=False)
                    nc.tensor.matmul(
                        ps, lhsT=CONS[0:1, 0:128],
                        rhs=VBS[:, vh * 384:(vh + 1) * 384],
                        start=False, stop=True)
                    psj = ps.rearrange("p (jh par h) -> p jh par h", par=2, h=64)
                    import concourse.bass as bass
                    vsrc = VALL[:, vh * 3:(vh + 1) * 3, tb, 0:64]
                    vdst = bass.AP(tensor=vsrc.tensor, offset=vsrc.offset,
                                   ap=[list(vsrc.ap[0]),
                                       [VW * NKB, 3], [129, 2], [1, 64]])
                    nc.scalar.copy(vdst, psj)

            wtv0 = w_p.tile([128, NC, 384], F32R, tag="wt", bufs=2)
            for c in range(NC):
                nc.sync.dma_start(
                    out=wtv0[:, c, :],
                    in_=wqkvT[c * 128:(c + 1) * 128, 2 * DIM:2 * DIM + 384])
            emit_v_half(0, wtv0, ps1_p, "ps1", 4)
            wtv1 = w_p.tile([128, NC, 384], F32R, tag="wt", bufs=2)
            for c in range(NC):
                nc.sync.dma_start(
                    out=wtv1[:, c, :],
                    in_=wqkvT[c * 128:(c + 1) * 128, 2 * DIM + 384:3 * DIM])
            emit_v_half(1, wtv1, ps1_p, "ps1", 4)

            for h in range(HEADS):
                rows = slice(64, 128) if h % 2 == 0 else slice(0, 64)
                nc.sync.dma_start(out=KALL[rows, h, :], in_=kconst)

        # ---------- 2: attention ----------
        aod_p = tc.alloc_tile_pool(name="aod", bufs=1)
        AOD = aod_p.tile([128, NC, N], F32R)
        w2a_p = tc.alloc_tile_pool(name="w2a", bufs=1)
        WP0 = w2a_p.tile([128, NC, 384], F32R)
        PBIAS = w2a_p.tile([128, NC], F32)
        for c in range(NC):
            nc.sync.dma_start(
                out=WP0[:, c, :], in_=wprojT[c * 128:(c + 1) * 128, 0:384])
        nc.sync.dma_start(
            out=PBIAS, in_=projb.rearrange("(c p) -> p c", p=128))
        with tc.tile_pool(name="pt", bufs=KNOB_PT_BUFS) as pt_p, \
             tc.tile_pool(name="sm", bufs=int(os.environ.get("K_SM","4"))) as sm_p, \
             tc.tile_pool(name="pss", bufs=int(os.environ.get("K_PSS","2")), space="PSUM") as psS_p, \
             tc.tile_pool(name="pspv", bufs=4, space="PSUM") as psPV_p:
            for head in range(HEADS):
                pair, par = head // 2, head % 2
                pv = [psPV_p.tile([128, 512], F32, tag="pv", bufs=int(os.environ.get("K_PV","4")), name=f"pv{head}_{qh}")
                      for qh in range(NQH)]
                vsl = (slice(0, 65) if par == 0 else slice(65, 193))
                for kb in range(NKB):
                    ps_s = psS_p.tile([128, 1024], F32, tag="pss")
                    for qh in range(NQH):
                        nc.tensor.matmul(
                            ps_s[:, qh * 512:(qh + 1) * 512],
                            lhsT=KALL[:, head, kb * 128:(kb + 1) * 128],
                            rhs=QALL[:, head, qh * 512:(qh + 1) * 512],
                            start=True, stop=True)
                    pt = pt_p.tile([128, 1024], BF16, tag="pt")
                    nc.scalar.activation(pt, ps_s, EXP)
                    for qh in range(NQH):
                        pv_out = pv[qh][0:65] if par == 0 else pv[qh]
                        nc.tensor.matmul(
                            pv_out, lhsT=VALL[:, pair, kb, vsl],
                            rhs=pt[:, qh * 512:(qh + 1) * 512],
                            start=(kb == 0), stop=(kb == NKB - 1))
                dr = 64 if par == 0 else 32     # denominator row (32-aligned)
                ao_rows = slice(0, 64) if par == 0 else slice(64, 128)
                for qh in range(NQH):
                    dsb = sm_p.tile([128, 512], F32R, tag="dsb",
                                    name=f"dsb{head}_{qh}")
                    nc.vector.tensor_copy(dsb[dr:dr + 1], pv[qh][dr:dr + 1])
                    rb = psPV_p.tile([128, 512], F32, tag="pv", bufs=int(os.environ.get("K_PV","4")),
                                     name=f"rb{head}_{qh}")
                    if par == 0:
                        nc.tensor.matmul(rb[0:64], lhsT=CONS[64:65, 0:64],
                                         rhs=dsb[64:65], start=True, stop=True)
                    else:
                        nc.tensor.matmul(rb, lhsT=CONS[32:33, 128:256],
                                         rhs=dsb[32:33], start=True, stop=True)
                    rbr = sm_p.tile([128, 512], F32, tag="rbr",
                                    name=f"rbr{head}_{qh}")
                    nc.vector.reciprocal(rbr[ao_rows], rb[ao_rows])
                    nc.vector.tensor_mul(
                        AOD[ao_rows, pair, qh * 512:(qh + 1) * 512],
                        pv[qh][ao_rows], rbr[ao_rows])

        # ---------- 3: proj + bias + transpose + out ----------
        with tc.tile_pool(name="wp", bufs=1) as w2_p, \
             tc.tile_pool(name="ps4", bufs=int(os.environ.get("K_PS4","4")), space="PSUM") as ps4_p:
            YSB = xt_p.tile([128, NC, N], F32, tag="xtslot")
            WP1 = w2_p.tile([128, NC, 384], F32R)
            for c in range(NC):
                nc.sync.dma_start(
                    out=WP1[:, c, :],
                    in_=wprojT[c * 128:(c + 1) * 128, 384:768])
            wt2s = [WP0, WP1]
            for ob in range(NC):
                for qh in range(NQH):
                    g, mi = ob // 3, ob % 3
                    ps = ps4_p.tile([128, 512], F32, tag="ps4",
                                    name=f"psp{qh}_{ob}")
                    for c in range(NC):
                        nc.tensor.matmul(
                            ps,
                            lhsT=wt2s[g][:, c, mi * 128:(mi + 1) * 128],
                            rhs=AOD[:, c, qh * 512:(qh + 1) * 512],
                            start=(c == 0), stop=(c == NC - 1))
                    nc.vector.tensor_scalar(
                        out=YSB[:, ob, qh * 512:(qh + 1) * 512], in0=ps,
                        scalar1=PBIAS[:, ob:ob + 1], scalar2=None, op0=ADD)
            for ob in range(NC):
                nc.sync.dma_start(out=y[ob * 128:(ob + 1) * 128, :],
                                  in_=YSB[:, ob, :])
        w2a_p.release()
        aod_p.release()
        xt_p.release()
        cons_p.release()
        vall_p.release()
        kall_p.release()
        qall_p.release()

    nc.compile()
    return nc


def host_prep(x, qkv_w, qkv_b, proj_w, proj_b, rel_pos_h, rel_pos_w):
    """full inputs -> list of 8 per-core in_maps"""
    x = np.asarray(x, np.float32)
    qkv_w = np.asarray(qkv_w, np.float32)
    qkv_b = np.asarray(qkv_b, np.float32)
    proj_w = np.asarray(proj_w, np.float32)
    proj_b = np.asarray(proj_b, np.float32)
    rel_pos_h = np.asarray(rel_pos_h, np.float32)
    rel_pos_w = np.asarray(rel_pos_w, np.float32)

    wqkvT = np.ascontiguousarray(qkv_w.T).copy()
    wqkvT[:, :DIM] *= SCALE
    qkvb2 = qkv_b.copy()
    qkvb2[:DIM] *= SCALE
    wprojT = np.ascontiguousarray(proj_w.T)

    idx = np.arange(H)
    Rh = rel_pos_h[idx[:, None] - idx[None, :] + (H - 1)]  # (32,32,64)
    Rw = rel_pos_w[idx[:, None] - idx[None, :] + (W - 1)]
    import ml_dtypes
    bdh = np.zeros((H, 128, 128), ml_dtypes.bfloat16)
    bdw = np.zeros((W, 128, 128), ml_dtypes.bfloat16)
    for h in range(H):
        bdh[h, 0:64, 64:96] = Rh[h].T / SCALE
        bdh[h, 64:128, 0:32] = Rh[h].T / SCALE
    for w in range(W):
        bdw[w, 0:64, 96:128] = Rw[w].T / SCALE
        bdw[w, 64:128, 32:64] = Rw[w].T / SCALE
    bdh = np.ascontiguousarray(bdh.transpose(1, 0, 2))  # [128, H, 128]
    bdw = np.ascontiguousarray(bdw.transpose(1, 0, 2))

    k = np.arange(N)
    kconst = np.zeros((64, N), np.float32)
    kconst[:32] = (k[None, :] // 32 == np.arange(32)[:, None])
    kconst[32:] = (k[None, :] % 32 == np.arange(32)[:, None])

    consd = np.zeros((128, 256), np.float32)
    consd[:, 0:128] = 1.0
    consd[:, 192:256] = 1.0
    vbrow = np.ascontiguousarray(qkvb2[2 * DIM:].reshape(1, DIM))

    shared = dict(wqkvT=wqkvT, qkvb=qkvb2, wprojT=wprojT, projb=proj_b,
                  bdh=bdh, bdw=bdw, kconst=kconst,
                  consd=consd, vbrow=vbrow)
    in_maps = []
    for b in range(B):
        xT = np.ascontiguousarray(x[b].reshape(N, DIM).T)
        in_maps.append(dict(xT=xT, **shared))
    return in_maps


def get_nc():
    if "nc" not in _CACHE:
        _CACHE["nc"] = build_nc()
    return _CACHE["nc"]


def kernel(**inputs):
    nc = get_nc()
    in_maps = host_prep(**inputs)
    res = bass_utils.run_bass_kernel_spmd(nc, in_maps, core_ids=list(range(NCORES)))
    out = np.stack([np.asarray(r["y"]).T for r in res.results], axis=0)
    return np.ascontiguousarray(out).reshape(B, H, W, DIM).astype(np.float32)



# revision 28
# speedup vs baseline: 1.1390x; 1.1390x over previous
"""Trainium2 Bass kernel: ViT-style global attention with decomposed
relative position bias (B=8, 32x32 tokens, dim 768, 12 heads, hd 64).

Sharding: data-parallel over batch B=8 -> one image per NeuronCore,
weights replicated, no collectives.

Fully software-pipelined single pass over 6 head-pairs:
  - production(P): q/k features (feature-major, fp32 PSUM chains over
    XT), V (token-major), rel-pos contraction per pair; emitted as PE
    "fillers" interleaved into the previous pair's attention so the PE
    never starves while ACT runs exp.
  - attention(head): per kb ONE K=128 matmul gives scale*S^T+bias in
    PSUM (bias rides contraction rows 64:128 vs indicator rows of K');
    exp on ACT -> PT (bf16). PV is token-major: lhsT = PT 128x128
    chunk, rhs = V[keys,65] (64 v-feats + ones col) -> out[tok,65]
    accumulated over kb; col 64 = softmax denominator for free.
  - cleanup(pair): reciprocal of D (DVE), divide via stride-0
    broadcast tensor_tensor (DVE), PE transpose back to feature-major
    AOD (bf16) for proj.
  - proj: fp32-accumulated bf16 matmuls + bias, y^T DMA'd out per
    chunk; host does the final transpose during unsharding.
Engine budget: PE ~135us (bound), ACT ~115us (exp), DVE ~60us,
Pool ~70us (drains).
"""

import os

import numpy as np

import concourse.bacc as bacc
import concourse.bass as bass
import concourse.tile as tile
from concourse import mybir
from concourse import bass_utils

B, H, W, DIM = 8, 32, 32, 768
HEADS, HD = 12, 64
N = H * W  # 1024
NCORES = 8
SCALE = HD ** -0.5
F32 = mybir.dt.float32
F32R = mybir.dt.float32r
BF16 = mybir.dt.bfloat16
EXP = mybir.ActivationFunctionType.Exp
IDN = mybir.ActivationFunctionType.Identity
ADD = mybir.AluOpType.add
MULT = mybir.AluOpType.mult

NC = DIM // 128      # 6 feature chunks == head pairs
NKB = N // 128       # 8 key/token blocks
NQH = N // 512       # 2 query halves

_CACHE = {}

NWARM = int(os.environ.get("K_WARM", "11"))
PT_BUFS = int(os.environ.get("K_PT", "16"))


def build_nc():
    nc = bacc.Bacc("TRN2", target_bir_lowering=False, debug=False)

    xT = nc.dram_tensor("xT", (DIM, N), BF16, kind="ExternalInput").ap()
    # packed per-pair weight columns: (pair, DIM, 384) = (q_p | k_p | v_p)
    wpack = nc.dram_tensor("wpack", (NC, DIM, 384), BF16, kind="ExternalInput").ap()
    qkvb = nc.dram_tensor("qkvb", (3 * DIM,), F32, kind="ExternalInput").ap()
    vbrow = nc.dram_tensor("vbrow", (1, DIM), BF16, kind="ExternalInput").ap()
    wprojT = nc.dram_tensor("wprojT", (DIM, DIM), BF16, kind="ExternalInput").ap()
    projb = nc.dram_tensor("projb", (DIM,), F32, kind="ExternalInput").ap()
    # rhwT: [64, H, 64] = (RhT | RwT) blocks; zeros64 fills the off-blocks
    rhwT = nc.dram_tensor("rhwT", (64, H, 64), BF16, kind="ExternalInput").ap()
    zeros64 = nc.dram_tensor("zeros64", (64, H, 64), BF16, kind="ExternalInput").ap()
    kconst = nc.dram_tensor("kconst", (64, N), BF16, kind="ExternalInput").ap()
    ident = nc.dram_tensor("ident", (128, 128), BF16, kind="ExternalInput").ap()
    y = nc.dram_tensor("y", (DIM, N), F32, kind="ExternalOutput").ap()

    qkvb2 = qkvb.rearrange("(c p one) -> c p one", p=128, one=1)   # [18][128,1]

    with tile.TileContext(nc) as tc:
        # ---- long-lived pools (bottom of SBUF stack) ----
        qall_p = tc.alloc_tile_pool(name="qall", bufs=1)
        kall_p = tc.alloc_tile_pool(name="kall", bufs=1)
        vall_p = tc.alloc_tile_pool(name="vall", bufs=1)
        cons_p = tc.alloc_tile_pool(name="cons", bufs=1)
        xt_p = tc.alloc_tile_pool(name="xtp", bufs=1)
        aod_p = tc.alloc_tile_pool(name="aod", bufs=1)

        QALL = qall_p.tile([128, HEADS, N], BF16)
        KALL = kall_p.tile([128, HEADS, N], BF16)
        VALL = vall_p.tile([128, HEADS, NKB, 65], BF16)
        STQB = cons_p.tile([128, NC, N], BF16)
        BD = cons_p.tile([128, H, 128], BF16)
        VBS = cons_p.tile([1, DIM], BF16)
        ONES1 = cons_p.tile([1, 128], BF16)
        IDT = cons_p.tile([128, 128], BF16)
        PBIAS = cons_p.tile([128, NC], F32)
        XT = xt_p.tile([128, NC, N], BF16, tag="xtslot")
        AOD = aod_p.tile([128, NC, N], BF16)

        with tc.tile_pool(name="wpool", bufs=1) as w_p, \
             tc.tile_pool(name="bias", bufs=4) as b_p, \
             tc.tile_pool(name="pt", bufs=PT_BUFS) as pt_p, \
             tc.tile_pool(name="stg", bufs=2) as stg_p, \
             tc.tile_pool(name="psA", bufs=2, space="PSUM") as psA_p, \
             tc.tile_pool(name="psB", bufs=2, space="PSUM") as psB_p:

            # ---------- constant / early DMAs ----------
            wts = {}

            def dma_pair_weights(p):
                wt = w_p.tile([128, NC, 384], BF16, tag="wt", bufs=2,
                              name=f"wt{p}")
                for c in range(NC):
                    nc.sync.dma_start(out=wt[:, c, :], in_=wpack[p, c * 128:(c + 1) * 128, :])
                wts[p] = wt

            def dma_pair_biases(p):
                qb = b_p.tile([128, 1], F32, tag="bias", bufs=6, name=f"qb{p}")
                nc.sync.dma_start(out=qb, in_=qkvb2[p])
                kb2 = b_p.tile([128, 1], F32, tag="bias", bufs=6, name=f"kb{p}")
                nc.sync.dma_start(out=kb2, in_=qkvb2[6 + p])
                return qb, kb2

            def dma_pair_kconst(p):
                nc.sync.dma_start(out=KALL[64:128, 2 * p, :], in_=kconst)
                nc.sync.dma_start(out=KALL[0:64, 2 * p + 1, :], in_=kconst)

            # PE p-state warm-up while DMAs land (memset first so DVE's later
            # big memsets don't gate the first warm matmul)
            if NWARM:
                jnk = cons_p.tile([128, 512], BF16)
                nc.vector.memset(jnk, 0.5)
                wps = psA_p.tile([128, 512], F32, tag="ps1", name="warmps")
                for _ in range(NWARM):
                    nc.tensor.matmul(wps, lhsT=jnk[:, 0:128], rhs=jnk,
                                     start=True, stop=True,
                                     skip_group_check=True)

            dma_pair_weights(0)
            dma_pair_kconst(0)
            for qh in range(NQH):
                for c in range(NC):
                    nc.sync.dma_start(
                        out=XT[:, c, qh * 512:(qh + 1) * 512],
                        in_=xT[c * 128:(c + 1) * 128, qh * 512:(qh + 1) * 512])
            biases = {0: dma_pair_biases(0)}
            # block-diagonal rel stationary: the four placements occupy
            # disjoint column ranges, so rel-h and rel-w share one tile (the
            # cross terms land in output rows that are never copied out).
            # Fully DMA'd (incl. zero blocks) to keep DVE off the lead-in.
            nc.sync.dma_start(out=BD[0:64, :, 64:128], in_=rhwT)
            nc.sync.dma_start(out=BD[64:128, :, 0:64], in_=rhwT)
            nc.sync.dma_start(out=BD[0:64, :, 0:64], in_=zeros64)
            nc.sync.dma_start(out=BD[64:128, :, 64:128], in_=zeros64)
            nc.sync.dma_start(out=VBS, in_=vbrow)
            nc.sync.dma_start(out=IDT, in_=ident)
            nc.vector.memset(ONES1, 1.0)
            nc.vector.memset(VALL.rearrange("p h k c -> p (h k) c")[:, :, 64:65], 1.0)

            # ---------- production of one pair (generator of PE pieces) ----
            stq4 = STQB.rearrange("p c (t ww) -> p c t ww", ww=W)

            def gen_production(p):
                """Yield rows_emitted after each small PE piece."""
                wt = wts[p]
                qb, kb2 = biases[p]
                he, ho = 2 * p, 2 * p + 1
                # q chains (feature-major)
                for qh in range(NQH):
                    qsl = slice(qh * 512, (qh + 1) * 512)
                    ps = psA_p.tile([128, 512], F32, tag="ps1",
                                    name=f"psq{p}_{qh}")
                    for c in range(NC):
                        nc.tensor.matmul(ps, lhsT=wt[:, c, 0:128],
                                         rhs=XT[:, c, qsl],
                                         start=(c == 0), stop=(c == NC - 1))
                        yield 512
                    nc.vector.tensor_scalar(out=QALL[0:64, he, qsl],
                                            in0=ps[0:64], scalar1=qb[0:64],
                                            scalar2=None, op0=ADD)
                    nc.vector.tensor_scalar(out=QALL[64:128, ho, qsl],
                                            in0=ps[64:128], scalar1=qb[64:128],
                                            scalar2=None, op0=ADD)
                    nc.vector.tensor_scalar(out=STQB[:, p, qsl], in0=ps,
                                            scalar1=qb, scalar2=None, op0=ADD)
                    yield 0
                # k chains
                for qh in range(NQH):
                    qsl = slice(qh * 512, (qh + 1) * 512)
                    ps = psA_p.tile([128, 512], F32, tag="ps1",
                                    name=f"psk{p}_{qh}")
                    for c in range(NC):
                        nc.tensor.matmul(ps, lhsT=wt[:, c, 128:256],
                                         rhs=XT[:, c, qsl],
                                         start=(c == 0), stop=(c == NC - 1))
                        yield 512
                    nc.vector.tensor_scalar(out=KALL[0:64, he, qsl],
                                            in0=ps[0:64], scalar1=kb2[0:64],
                                            scalar2=None, op0=ADD)
                    nc.vector.tensor_scalar(out=KALL[64:128, ho, qsl],
                                            in0=ps[64:128], scalar1=kb2[64:128],
                                            scalar2=None, op0=ADD)
                    yield 0
                # rel-pos: h then w, in two 16-row halves
                for half in range(2):
                    hsl = slice(half * 512, (half + 1) * 512)
                    psr = psA_p.tile([128, 16, 32], F32, tag="ps1",
                                     name=f"psrh{p}_{half}")
                    for j in range(16):
                        h = half * 16 + j
                        nc.tensor.matmul(psr[:, j, :], lhsT=BD[:, h, :],
                                         rhs=STQB[:, p, h * 32:(h + 1) * 32],
                                         start=True, stop=True,
                                         skip_group_check=True)
                        if j % 4 == 3:
                            yield 128
                    nc.vector.tensor_copy(
                        QALL[64:96, he, hsl].rearrange("p (hb t) -> p hb t", t=32),
                        psr[64:96])
                    nc.vector.tensor_copy(
                        QALL[0:32, ho, hsl].rearrange("p (hb t) -> p hb t", t=32),
                        psr[0:32])
                    yield 0
                qvwE = QALL[96:128, he, :].rearrange("p (t wb) -> p wb t", wb=32)
                qvwO = QALL[32:64, ho, :].rearrange("p (t wb) -> p wb t", wb=32)
                for half in range(2):
                    psr = psA_p.tile([128, 16, 32], F32, tag="ps1",
                                     name=f"psrw{p}_{half}")
                    for j in range(16):
                        w = half * 16 + j
                        nc.tensor.matmul(psr[:, j, :], lhsT=BD[:, w, :],
                                         rhs=stq4[:, p, :, w],
                                         start=True, stop=True,
                                         skip_group_check=True)
                        if j % 4 == 3:
                            yield 128
                    nc.vector.tensor_copy(qvwE[:, half * 16:(half + 1) * 16, :],
                                          psr[96:128])
                    nc.vector.tensor_copy(qvwO[:, half * 16:(half + 1) * 16, :],
                                          psr[32:64])
                    yield 0

                # v chains (token-major), 2 token-blocks per psum tile,
                # bias via K=1 matmul, single ACT drain per tile
                for vb in range(4):
                    psv = psA_p.tile([128, 2, 128], F32, tag="ps1",
                                     name=f"psv{p}_{vb}")
                    for t2 in range(2):
                        tb = vb * 2 + t2
                        for c in range(NC):
                            nc.tensor.matmul(
                                psv[:, t2, :],
                                lhsT=XT[:, c, tb * 128:(tb + 1) * 128],
                                rhs=wt[:, c, 256:384],
                                start=(c == 0), stop=False,
                                skip_group_check=True)
                            yield 128
                        nc.tensor.matmul(psv[:, t2, :],
                                         lhsT=ONES1[0:1, 0:128],
                                         rhs=VBS[:, p * 128:(p + 1) * 128],
                                         start=False, stop=True,
                                         skip_group_check=True)
                        yield 128
                    nc.scalar.copy(
                        VALL[:, he:ho + 1, vb * 2:(vb + 1) * 2, 0:64],
                        psv.rearrange("p t (hh f) -> p hh t f", hh=2))
                    yield 0
            # ---------- attention pieces ----------
            def emit_qk_exp(h, kb):
                ps_s = psA_p.tile([128, N], F32, tag="s", name=f"s{h}_{kb}")
                for qh in range(NQH):
                    nc.tensor.matmul(
                        ps_s[:, qh * 512:(qh + 1) * 512],
                        lhsT=KALL[:, h, kb * 128:(kb + 1) * 128],
                        rhs=QALL[:, h, qh * 512:(qh + 1) * 512],
                        start=True, stop=True)
                pt = pt_p.tile([128, N], BF16, tag="pt", name=f"pt{h}_{kb}")
                nc.scalar.activation(pt, ps_s, EXP)
                return pt

            def emit_pv(h, pts, aodt):
                """PV in two 4-token-block halves; the softmax divide is
                fused into the drain: one reciprocal per half + one
                stride-0-broadcast multiply PSUM->aodt."""
                par = h % 2
                fsl = slice(par * 64, par * 64 + 64)
                for half in range(2):
                    pv = psB_p.tile([128, 4, 128], F32, tag="pv",
                                    name=f"pv{h}_{half}")
                    for t2 in range(4):
                        tb = half * 4 + t2
                        for kb in range(NKB):
                            nc.tensor.matmul(
                                pv[:, t2, 0:65],
                                lhsT=pts[kb][:, tb * 128:(tb + 1) * 128],
                                rhs=VALL[:, h, kb, :],
                                start=(kb == 0), stop=(kb == NKB - 1),
                                skip_group_check=True)
                    rh = stg_p.tile([128, 4], F32, tag="rd", bufs=4,
                                    name=f"rd{h}_{half}")
                    nc.vector.reciprocal(
                        rh, pv[:, :, 64:65].rearrange("p t one -> p (t one)"))
                    in1 = bass.AP(tensor=rh.tensor, offset=rh.offset,
                                  ap=[list(rh.ap[0]), [1, 4], [0, 64]])
                    nc.vector.tensor_mul(
                        aodt[:, half * 4:(half + 1) * 4, fsl],
                        pv[:, :, 0:64], in1)

            def emit_cleanup(p, aodt):
                for tp in range(4):
                    pst = psA_p.tile([128, 2, 128], BF16, tag="ps1",
                                     name=f"pst{p}_{tp}")
                    for t2 in range(2):
                        nc.tensor.transpose(pst[:, t2, :],
                                            aodt[:, tp * 2 + t2, :], IDT)
                    nc.vector.tensor_copy(
                        AOD[:, p, tp * 256:(tp + 1) * 256],
                        pst.rearrange("p a b -> p (a b)"))

            # ---------- the pipelined main loop ----------
            fillers = None      # generator producing pair p+1
            prev_pts = None     # PT tiles of previous head
            prev_aodt = None    # token-major attention-out of prev head's pair

            # prologue: produce pair 0 outright
            for _ in gen_production(0):
                pass
            dma_pair_weights(1)
            biases[1] = dma_pair_biases(1)
            dma_pair_kconst(1)
            fillers = gen_production(1)

            PAIR_ROWS = 21504.0  # PE rows per pair production

            aodts = {}
            for j in range(HEADS):
                h = j
                p = j // 2
                par = j % 2
                if par == 0:
                    aodts[p] = stg_p.tile([128, NKB, 128], BF16, tag="aodt",
                                          name=f"aodt{p}")
                    # DMAs for pair p+2 production (consumed via fillers at
                    # steps 2p+2, 2p+3)
                    if p + 2 < NC:
                        dma_pair_weights(p + 2)
                        biases[p + 2] = dma_pair_biases(p + 2)
                        dma_pair_kconst(p + 2)
                    if p == 2:
                        WP = cons_p.tile([128, NC, DIM], BF16)
                        for c in range(NC):
                            nc.sync.dma_start(
                                out=WP[:, c, :],
                                in_=wprojT[c * 128:(c + 1) * 128, :])
                        nc.sync.dma_start(
                            out=PBIAS, in_=projb.rearrange("(c p) -> p c", p=128))
                    # production of pair p+1 interleaves into this pair's steps
                    if fillers is None and p + 1 < NC:
                        fillers = gen_production(p + 1)

                # QK + exp for head h, pulling fillers to keep PE fed.
                # Front-load q/k/rel of the next pair into the even head's
                # slots so the next pair's QK is never production-gated;
                # only v (needed a step later) rides the odd head's slots.
                pts = []
                budget = 0.0
                for kb in range(NKB):
                    pts.append(emit_qk_exp(h, kb))
                    budget += (14336.0 if par == 0 else 7168.0) / 8.0
                    while fillers is not None and budget > 0:
                        try:
                            budget -= next(fillers)
                        except StopIteration:
                            fillers = None
                # cleanup of pair p-1: its last PV was emitted in the
                # previous step; deferring to after this step's QK loop gives
                # the DVE recip/divide chain a full QK window to complete
                # before the PE reaches the transposes.
                if par == 1 and p >= 1:
                    emit_cleanup(p - 1, aodts.pop(p - 1))

                # PV of the previous head
                if prev_pts is not None:
                    emit_pv(h - 1, prev_pts, prev_aodt)
                prev_pts, prev_aodt = pts, aodts[p]

                # drain any residual production at pair boundaries
                if par == 1 and fillers is not None:
                    for _ in fillers:
                        pass
                    fillers = None

            emit_pv(HEADS - 1, prev_pts, prev_aodt)
            emit_cleanup(NC - 1, aodts.pop(NC - 1))

        # ---------- proj + bias + out ----------
        with tc.tile_pool(name="ps4", bufs=6, space="PSUM") as ps4_p, \
             tc.tile_pool(name="wpp", bufs=1) as wp2_p:
            YSB = xt_p.tile([128, NC, N], F32, tag="xtslot")
            for ob in range(NC):
                for qh in range(NQH):
                    qsl = slice(qh * 512, (qh + 1) * 512)
                    ps = ps4_p.tile([128, 512], F32, tag="ps4",
                                    name=f"psp{ob}_{qh}")
                    for c in range(NC):
                        nc.tensor.matmul(
                            ps, lhsT=WP[:, c, ob * 128:(ob + 1) * 128],
                            rhs=AOD[:, c, qsl],
                            start=(c == 0), stop=(c == NC - 1))
                    nc.vector.tensor_scalar(
                        out=YSB[:, ob, qsl], in0=ps,
                        scalar1=PBIAS[:, ob:ob + 1], scalar2=None, op0=ADD)
                nc.sync.dma_start(out=y[ob * 128:(ob + 1) * 128, :],
                                  in_=YSB[:, ob, :])
        aod_p.release()
        xt_p.release()
        cons_p.release()
        vall_p.release()
        kall_p.release()
        qall_p.release()

    nc.compile()
    return nc


def host_prep(x, qkv_w, qkv_b, proj_w, proj_b, rel_pos_h, rel_pos_w):
    """full inputs -> list of 8 per-core in_maps"""
    import ml_dtypes
    x = np.asarray(x, np.float32)
    qkv_w = np.asarray(qkv_w, np.float32)
    qkv_b = np.asarray(qkv_b, np.float32)
    proj_w = np.asarray(proj_w, np.float32)
    proj_b = np.asarray(proj_b, np.float32)
    rel_pos_h = np.asarray(rel_pos_h, np.float32)
    rel_pos_w = np.asarray(rel_pos_w, np.float32)

    wqkvT = np.ascontiguousarray(qkv_w.T).copy()   # (768, 2304)
    wqkvT[:, :DIM] *= SCALE
    qkvb2 = qkv_b.copy()
    qkvb2[:DIM] *= SCALE
    # packed (pair, 768, 384) = q_p | k_p | v_p
    wpack = np.empty((NC, DIM, 384), np.float32)
    for p in range(NC):
        wpack[p, :, 0:128] = wqkvT[:, p * 128:(p + 1) * 128]
        wpack[p, :, 128:256] = wqkvT[:, DIM + p * 128:DIM + (p + 1) * 128]
        wpack[p, :, 256:384] = wqkvT[:, 2 * DIM + p * 128:2 * DIM + (p + 1) * 128]
    wpack = wpack.astype(ml_dtypes.bfloat16)
    wprojT = np.ascontiguousarray(proj_w.T).astype(ml_dtypes.bfloat16)

    idx = np.arange(H)
    Rh = rel_pos_h[idx[:, None] - idx[None, :] + (H - 1)]  # (32,32,64) [q,k,c]
    Rw = rel_pos_w[idx[:, None] - idx[None, :] + (W - 1)]
    rhwT = np.concatenate(
        [Rh.transpose(2, 0, 1) / SCALE, Rw.transpose(2, 0, 1) / SCALE],
        axis=2)  # (64, 32, 64)
    rhwT = np.ascontiguousarray(rhwT).astype(ml_dtypes.bfloat16)
    zeros64 = np.zeros((64, H, 64), ml_dtypes.bfloat16)

    k = np.arange(N)
    kconst = np.zeros((64, N), np.float32)
    kconst[:32] = (k[None, :] // 32 == np.arange(32)[:, None])
    kconst[32:] = (k[None, :] % 32 == np.arange(32)[:, None])
    kconst = kconst.astype(ml_dtypes.bfloat16)

    ident = np.eye(128, dtype=ml_dtypes.bfloat16)
    vbrow = np.ascontiguousarray(qkvb2[2 * DIM:].reshape(1, DIM)).astype(ml_dtypes.bfloat16)

    shared = dict(wpack=wpack, qkvb=qkvb2, wprojT=wprojT, projb=proj_b,
                  rhwT=rhwT, zeros64=zeros64, kconst=kconst, ident=ident,
                  vbrow=vbrow)
    in_maps = []
    for b in range(B):
        xTb = np.ascontiguousarray(x[b].reshape(N, DIM).T).astype(ml_dtypes.bfloat16)
        in_maps.append(dict(xT=xTb, **shared))
    return in_maps


def get_nc():
    if "nc" not in _CACHE:
        _CACHE["nc"] = build_nc()
    return _CACHE["nc"]


def kernel(**inputs):
    nc = get_nc()
    in_maps = host_prep(**inputs)
    res = bass_utils.run_bass_kernel_spmd(nc, in_maps, core_ids=list(range(NCORES)))
    out = np.stack([np.asarray(r["y"]).T for r in res.results], axis=0)
    return np.ascontiguousarray(out).reshape(B, H, W, DIM).astype(np.float32)


# revision 30
# speedup vs baseline: 1.1615x; 1.0198x over previous
"""Trainium2 Bass kernel: ViT-style global attention with decomposed
relative position bias (B=8, 32x32 tokens, dim 768, 12 heads, hd 64).

Sharding: data-parallel over batch B=8 -> one image per NeuronCore,
weights replicated, no collectives.

Fully software-pipelined single pass over 6 head-pairs:
  - production(P): q/k features (feature-major, fp32 PSUM chains over
    XT), V (token-major), rel-pos contraction per pair; emitted as PE
    "fillers" interleaved into the previous pair's attention so the PE
    never starves while ACT runs exp.
  - attention(head): per kb ONE K=128 matmul gives scale*S^T+bias in
    PSUM (bias rides contraction rows 64:128 vs indicator rows of K');
    exp on ACT -> PT (bf16). PV is token-major: lhsT = PT 128x128
    chunk, rhs = V[keys,65] (64 v-feats + ones col) -> out[tok,65]
    accumulated over kb; col 64 = softmax denominator for free.
  - cleanup(pair): reciprocal of D (DVE), divide via stride-0
    broadcast tensor_tensor (DVE), PE transpose back to feature-major
    AOD (bf16) for proj.
  - proj: fp32-accumulated bf16 matmuls + bias, y^T DMA'd out per
    chunk; host does the final transpose during unsharding.
Engine budget: PE ~135us (bound), ACT ~115us (exp), DVE ~60us,
Pool ~70us (drains).
"""

import os

import numpy as np

import concourse.bacc as bacc
import concourse.bass as bass
import concourse.tile as tile
from concourse import mybir
from concourse import bass_utils

B, H, W, DIM = 8, 32, 32, 768
HEADS, HD = 12, 64
N = H * W  # 1024
NCORES = 8
SCALE = HD ** -0.5
F32 = mybir.dt.float32
F32R = mybir.dt.float32r
BF16 = mybir.dt.bfloat16
EXP = mybir.ActivationFunctionType.Exp
IDN = mybir.ActivationFunctionType.Identity
ADD = mybir.AluOpType.add
MULT = mybir.AluOpType.mult

NC = DIM // 128      # 6 feature chunks == head pairs
NKB = N // 128       # 8 key/token blocks
NQH = N // 512       # 2 query halves

_CACHE = {}

NWARM = int(os.environ.get("K_WARM", "11"))
PT_BUFS = int(os.environ.get("K_PT", "16"))


def build_nc():
    nc = bacc.Bacc("TRN2", target_bir_lowering=False, debug=False)

    xT = nc.dram_tensor("xT", (DIM, N), BF16, kind="ExternalInput").ap()
    # packed per-pair weight columns: (pair, DIM, 384) = (q_p | k_p | v_p)
    wpack = nc.dram_tensor("wpack", (NC, DIM, 384), BF16, kind="ExternalInput").ap()
    qkvb = nc.dram_tensor("qkvb", (3 * DIM,), F32, kind="ExternalInput").ap()
    wprojT = nc.dram_tensor("wprojT", (DIM, DIM), BF16, kind="ExternalInput").ap()
    projb = nc.dram_tensor("projb", (DIM,), F32, kind="ExternalInput").ap()
    # rhwT: [64, H, 64] = (RhT | RwT) blocks; zeros64 fills the off-blocks
    rhwT = nc.dram_tensor("rhwT", (64, H, 64), BF16, kind="ExternalInput").ap()
    zeros64 = nc.dram_tensor("zeros64", (64, H, 64), BF16, kind="ExternalInput").ap()
    kconst = nc.dram_tensor("kconst", (64, N), BF16, kind="ExternalInput").ap()
    ident = nc.dram_tensor("ident", (128, 128), BF16, kind="ExternalInput").ap()
    y = nc.dram_tensor("y", (DIM, N), F32, kind="ExternalOutput").ap()

    qkvb2 = qkvb.rearrange("(c p one) -> c p one", p=128, one=1)   # [18][128,1]

    with tile.TileContext(nc) as tc:
        # ---- long-lived pools (bottom of SBUF stack) ----
        qall_p = tc.alloc_tile_pool(name="qall", bufs=1)
        kall_p = tc.alloc_tile_pool(name="kall", bufs=1)
        vall_p = tc.alloc_tile_pool(name="vall", bufs=1)
        cons_p = tc.alloc_tile_pool(name="cons", bufs=1)
        xt_p = tc.alloc_tile_pool(name="xtp", bufs=1)
        aod_p = tc.alloc_tile_pool(name="aod", bufs=1)

        QALL = qall_p.tile([128, HEADS, N], BF16)
        KALL = kall_p.tile([128, HEADS, N], BF16)
        VALL = vall_p.tile([128, HEADS, NKB, 65], BF16)
        STQB = cons_p.tile([128, NC, N], BF16)
        BD = cons_p.tile([128, H, 128], BF16)
        IDT = cons_p.tile([128, 128], BF16)
        PBIAS = cons_p.tile([128, NC], F32)
        XT = xt_p.tile([128, NC, N], BF16, tag="xtslot")
        AOD = aod_p.tile([128, NC, N], BF16)

        with tc.tile_pool(name="wpool", bufs=1) as w_p, \
             tc.tile_pool(name="bias", bufs=4) as b_p, \
             tc.tile_pool(name="pt", bufs=PT_BUFS) as pt_p, \
             tc.tile_pool(name="stg", bufs=2) as stg_p, \
             tc.tile_pool(name="psA", bufs=2, space="PSUM") as psA_p, \
             tc.tile_pool(name="psB", bufs=2, space="PSUM") as psB_p:

            # ---------- constant / early DMAs ----------
            wts = {}

            def dma_pair_weights(p):
                wt = w_p.tile([128, NC, 384], BF16, tag="wt", bufs=2,
                              name=f"wt{p}")
                for c in range(NC):
                    nc.sync.dma_start(out=wt[:, c, :], in_=wpack[p, c * 128:(c + 1) * 128, :])
                wts[p] = wt

            def dma_pair_biases(p):
                qb = b_p.tile([128, 1], F32, tag="bias", bufs=6, name=f"qb{p}")
                nc.sync.dma_start(out=qb, in_=qkvb2[p])
                kb2 = b_p.tile([128, 1], F32, tag="bias", bufs=6, name=f"kb{p}")
                nc.sync.dma_start(out=kb2, in_=qkvb2[6 + p])
                return qb, kb2

            def dma_pair_kconst(p):
                nc.sync.dma_start(out=KALL[64:128, 2 * p, :], in_=kconst)
                nc.sync.dma_start(out=KALL[0:64, 2 * p + 1, :], in_=kconst)

            # PE p-state warm-up while DMAs land (memset first so DVE's later
            # big memsets don't gate the first warm matmul)
            if NWARM:
                jnk = cons_p.tile([128, 512], BF16)
                nc.vector.memset(jnk, 0.5)
                wps = psA_p.tile([128, 512], F32, tag="ps1", name="warmps")
                for _ in range(NWARM):
                    nc.tensor.matmul(wps, lhsT=jnk[:, 0:128], rhs=jnk,
                                     start=True, stop=True,
                                     skip_group_check=True)

            dma_pair_weights(0)
            dma_pair_kconst(0)
            for qh in range(NQH):
                for c in range(NC):
                    nc.sync.dma_start(
                        out=XT[:, c, qh * 512:(qh + 1) * 512],
                        in_=xT[c * 128:(c + 1) * 128, qh * 512:(qh + 1) * 512])
            biases = {0: dma_pair_biases(0)}
            # block-diagonal rel stationary: the four placements occupy
            # disjoint column ranges, so rel-h and rel-w share one tile (the
            # cross terms land in output rows that are never copied out).
            # Fully DMA'd (incl. zero blocks) to keep DVE off the lead-in.
            nc.sync.dma_start(out=BD[0:64, :, 64:128], in_=rhwT)
            nc.sync.dma_start(out=BD[64:128, :, 0:64], in_=rhwT)
            nc.sync.dma_start(out=BD[0:64, :, 0:64], in_=zeros64)
            nc.sync.dma_start(out=BD[64:128, :, 64:128], in_=zeros64)
            nc.sync.dma_start(out=IDT, in_=ident)
            nc.vector.memset(VALL.rearrange("p h k c -> p (h k) c")[:, :, 64:65], 1.0)

            # ---------- production of one pair (generator of PE pieces) ----
            stq4 = STQB.rearrange("p c (t ww) -> p c t ww", ww=W)

            def gen_production(p):
                """Yield rows_emitted after each small PE piece."""
                wt = wts[p]
                qb, kb2 = biases[p]
                he, ho = 2 * p, 2 * p + 1
                # q chains (feature-major)
                for qh in range(NQH):
                    qsl = slice(qh * 512, (qh + 1) * 512)
                    ps = psA_p.tile([128, 512], F32, tag="ps1",
                                    name=f"psq{p}_{qh}")
                    for c in range(NC):
                        nc.tensor.matmul(ps, lhsT=wt[:, c, 0:128],
                                         rhs=XT[:, c, qsl],
                                         start=(c == 0), stop=(c == NC - 1))
                        yield 512
                    nc.vector.tensor_scalar(out=QALL[0:64, he, qsl],
                                            in0=ps[0:64], scalar1=qb[0:64],
                                            scalar2=None, op0=ADD)
                    nc.vector.tensor_scalar(out=QALL[64:128, ho, qsl],
                                            in0=ps[64:128], scalar1=qb[64:128],
                                            scalar2=None, op0=ADD)
                    nc.vector.tensor_scalar(out=STQB[:, p, qsl], in0=ps,
                                            scalar1=qb, scalar2=None, op0=ADD)
                    yield 0
                # k chains
                for qh in range(NQH):
                    qsl = slice(qh * 512, (qh + 1) * 512)
                    ps = psA_p.tile([128, 512], F32, tag="ps1",
                                    name=f"psk{p}_{qh}")
                    for c in range(NC):
                        nc.tensor.matmul(ps, lhsT=wt[:, c, 128:256],
                                         rhs=XT[:, c, qsl],
                                         start=(c == 0), stop=(c == NC - 1))
                        yield 512
                    nc.vector.tensor_scalar(out=KALL[0:64, he, qsl],
                                            in0=ps[0:64], scalar1=kb2[0:64],
                                            scalar2=None, op0=ADD)
                    nc.vector.tensor_scalar(out=KALL[64:128, ho, qsl],
                                            in0=ps[64:128], scalar1=kb2[64:128],
                                            scalar2=None, op0=ADD)
                    yield 0
                # rel-pos: h then w, in two 16-row halves
                for half in range(2):
                    hsl = slice(half * 512, (half + 1) * 512)
                    psr = psA_p.tile([128, 16, 32], F32, tag="ps1",
                                     name=f"psrh{p}_{half}")
                    for j in range(16):
                        h = half * 16 + j
                        nc.tensor.matmul(psr[:, j, :], lhsT=BD[:, h, :],
                                         rhs=STQB[:, p, h * 32:(h + 1) * 32],
                                         start=True, stop=True,
                                         skip_group_check=True)
                        if j % 4 == 3:
                            yield 128
                    nc.vector.tensor_copy(
                        QALL[64:96, he, hsl].rearrange("p (hb t) -> p hb t", t=32),
                        psr[64:96])
                    nc.vector.tensor_copy(
                        QALL[0:32, ho, hsl].rearrange("p (hb t) -> p hb t", t=32),
                        psr[0:32])
                    yield 0
                qvwE = QALL[96:128, he, :].rearrange("p (t wb) -> p wb t", wb=32)
                qvwO = QALL[32:64, ho, :].rearrange("p (t wb) -> p wb t", wb=32)
                for half in range(2):
                    psr = psA_p.tile([128, 16, 32], F32, tag="ps1",
                                     name=f"psrw{p}_{half}")
                    for j in range(16):
                        w = half * 16 + j
                        nc.tensor.matmul(psr[:, j, :], lhsT=BD[:, w, :],
                                         rhs=stq4[:, p, :, w],
                                         start=True, stop=True,
                                         skip_group_check=True)
                        if j % 4 == 3:
                            yield 128
                    nc.vector.tensor_copy(qvwE[:, half * 16:(half + 1) * 16, :],
                                          psr[96:128])
                    nc.vector.tensor_copy(qvwO[:, half * 16:(half + 1) * 16, :],
                                          psr[32:64])
                    yield 0

                # v chains (token-major), 2 token-blocks per psum tile,
                # single ACT drain per tile. v-bias is folded into the proj
                # bias on the host (y is affine in v).
                for vb in range(4):
                    psv = psA_p.tile([128, 2, 128], F32, tag="ps1",
                                     name=f"psv{p}_{vb}")
                    for t2 in range(2):
                        tb = vb * 2 + t2
                        for c in range(NC):
                            nc.tensor.matmul(
                                psv[:, t2, :],
                                lhsT=XT[:, c, tb * 128:(tb + 1) * 128],
                                rhs=wt[:, c, 256:384],
                                start=(c == 0), stop=(c == NC - 1),
                                skip_group_check=True)
                            yield 128
                    nc.scalar.copy(
                        VALL[:, he:ho + 1, vb * 2:(vb + 1) * 2, 0:64],
                        psv.rearrange("p t (hh f) -> p hh t f", hh=2))
                    yield 0
            # ---------- attention pieces ----------
            def emit_qk_exp(h, kb):
                ps_s = psA_p.tile([128, N], F32, tag="s", name=f"s{h}_{kb}")
                for qh in range(NQH):
                    nc.tensor.matmul(
                        ps_s[:, qh * 512:(qh + 1) * 512],
                        lhsT=KALL[:, h, kb * 128:(kb + 1) * 128],
                        rhs=QALL[:, h, qh * 512:(qh + 1) * 512],
                        start=True, stop=True)
                pt = pt_p.tile([128, N], BF16, tag="pt", name=f"pt{h}_{kb}")
                nc.scalar.activation(pt, ps_s, EXP)
                return pt

            def emit_pv(h, pts, aodt):
                """PV in two 4-token-block halves; the softmax divide is
                fused into the drain: one reciprocal per half + one
                stride-0-broadcast multiply PSUM->aodt."""
                par = h % 2
                fsl = slice(par * 64, par * 64 + 64)
                for half in range(2):
                    pv = psB_p.tile([128, 4, 128], F32, tag="pv",
                                    name=f"pv{h}_{half}")
                    for t2 in range(4):
                        tb = half * 4 + t2
                        for kb in range(NKB):
                            nc.tensor.matmul(
                                pv[:, t2, 0:65],
                                lhsT=pts[kb][:, tb * 128:(tb + 1) * 128],
                                rhs=VALL[:, h, kb, :],
                                start=(kb == 0), stop=(kb == NKB - 1),
                                skip_group_check=True)
                    rh = stg_p.tile([128, 4], F32, tag="rd", bufs=4,
                                    name=f"rd{h}_{half}")
                    nc.vector.reciprocal(
                        rh, pv[:, :, 64:65].rearrange("p t one -> p (t one)"))
                    in1 = bass.AP(tensor=rh.tensor, offset=rh.offset,
                                  ap=[list(rh.ap[0]), [1, 4], [0, 64]])
                    nc.vector.tensor_mul(
                        aodt[:, half * 4:(half + 1) * 4, fsl],
                        pv[:, :, 0:64], in1)

            def emit_cleanup(p, aodt):
                for tp in range(4):
                    pst = psA_p.tile([128, 2, 128], BF16, tag="ps1",
                                     name=f"pst{p}_{tp}")
                    for t2 in range(2):
                        nc.tensor.transpose(pst[:, t2, :],
                                            aodt[:, tp * 2 + t2, :], IDT)
                    nc.vector.tensor_copy(
                        AOD[:, p, tp * 256:(tp + 1) * 256],
                        pst.rearrange("p a b -> p (a b)"))

            # ---------- the pipelined main loop ----------
            fillers = None      # generator producing pair p+1
            prev_pts = None     # PT tiles of previous head
            prev_aodt = None    # token-major attention-out of prev head's pair

            # prologue: produce pair 0 outright
            for _ in gen_production(0):
                pass
            dma_pair_weights(1)
            biases[1] = dma_pair_biases(1)
            dma_pair_kconst(1)
            fillers = gen_production(1)

            PAIR_ROWS = 20480.0  # PE rows per pair production

            aodts = {}
            for j in range(HEADS):
                h = j
                p = j // 2
                par = j % 2
                if par == 0:
                    aodts[p] = stg_p.tile([128, NKB, 128], BF16, tag="aodt",
                                          name=f"aodt{p}")
                    # DMAs for pair p+2 production (consumed via fillers at
                    # steps 2p+2, 2p+3)
                    if p + 2 < NC:
                        dma_pair_weights(p + 2)
                        biases[p + 2] = dma_pair_biases(p + 2)
                        dma_pair_kconst(p + 2)
                    if p == 2:
                        WP = cons_p.tile([128, NC, DIM], BF16)
                        for c in range(NC):
                            nc.sync.dma_start(
                                out=WP[:, c, :],
                                in_=wprojT[c * 128:(c + 1) * 128, :])
                        nc.sync.dma_start(
                            out=PBIAS, in_=projb.rearrange("(c p) -> p c", p=128))
                    # production of pair p+1 interleaves into this pair's steps
                    if fillers is None and p + 1 < NC:
                        fillers = gen_production(p + 1)

                # QK + exp for head h, pulling fillers to keep PE fed.
                # Front-load q/k/rel of the next pair into the even head's
                # slots so the next pair's QK is never production-gated;
                # only v (needed a step later) rides the odd head's slots.
                pts = []
                budget = 0.0
                for kb in range(NKB):
                    pts.append(emit_qk_exp(h, kb))
                    budget += (14336.0 if par == 0 else 6144.0) / 8.0
                    while fillers is not None and budget > 0:
                        try:
                            budget -= next(fillers)
                        except StopIteration:
                            fillers = None
                # cleanup of pair p-1: its last PV was emitted in the
                # previous step; deferring to after this step's QK loop gives
                # the DVE recip/divide chain a full QK window to complete
                # before the PE reaches the transposes.
                if par == 1 and p >= 1:
                    emit_cleanup(p - 1, aodts.pop(p - 1))

                # PV of the previous head
                if prev_pts is not None:
                    emit_pv(h - 1, prev_pts, prev_aodt)
                prev_pts, prev_aodt = pts, aodts[p]

                # drain any residual production at pair boundaries
                if par == 1 and fillers is not None:
                    for _ in fillers:
                        pass
                    fillers = None

            emit_pv(HEADS - 1, prev_pts, prev_aodt)
            emit_cleanup(NC - 1, aodts.pop(NC - 1))

        # ---------- proj + bias + out ----------
        with tc.tile_pool(name="ps4", bufs=6, space="PSUM") as ps4_p, \
             tc.tile_pool(name="wpp", bufs=1) as wp2_p:
            YSB = xt_p.tile([128, NC, N], F32, tag="xtslot")
            for ob in range(NC):
                for qh in range(NQH):
                    qsl = slice(qh * 512, (qh + 1) * 512)
                    ps = ps4_p.tile([128, 512], F32, tag="ps4",
                                    name=f"psp{ob}_{qh}")
                    for c in range(NC):
                        nc.tensor.matmul(
                            ps, lhsT=WP[:, c, ob * 128:(ob + 1) * 128],
                            rhs=AOD[:, c, qsl],
                            start=(c == 0), stop=(c == NC - 1))
                    nc.vector.tensor_scalar(
                        out=YSB[:, ob, qsl], in0=ps,
                        scalar1=PBIAS[:, ob:ob + 1], scalar2=None, op0=ADD)
                    nc.sync.dma_start(out=y[ob * 128:(ob + 1) * 128, qsl],
                                      in_=YSB[:, ob, qsl])
        aod_p.release()
        xt_p.release()
        cons_p.release()
        vall_p.release()
        kall_p.release()
        qall_p.release()

    nc.compile()
    return nc


def host_prep(x, qkv_w, qkv_b, proj_w, proj_b, rel_pos_h, rel_pos_w):
    """full inputs -> list of 8 per-core in_maps"""
    import ml_dtypes
    x = np.asarray(x, np.float32)
    qkv_w = np.asarray(qkv_w, np.float32)
    qkv_b = np.asarray(qkv_b, np.float32)
    proj_w = np.asarray(proj_w, np.float32)
    proj_b = np.asarray(proj_b, np.float32)
    rel_pos_h = np.asarray(rel_pos_h, np.float32)
    rel_pos_w = np.asarray(rel_pos_w, np.float32)

    wqkvT = np.ascontiguousarray(qkv_w.T).copy()   # (768, 2304)
    wqkvT[:, :DIM] *= SCALE
    qkvb2 = qkv_b.copy()
    qkvb2[:DIM] *= SCALE
    # packed (pair, 768, 384) = q_p | k_p | v_p
    wpack = np.empty((NC, DIM, 384), np.float32)
    for p in range(NC):
        wpack[p, :, 0:128] = wqkvT[:, p * 128:(p + 1) * 128]
        wpack[p, :, 128:256] = wqkvT[:, DIM + p * 128:DIM + (p + 1) * 128]
        wpack[p, :, 256:384] = wqkvT[:, 2 * DIM + p * 128:2 * DIM + (p + 1) * 128]
    wpack = wpack.astype(ml_dtypes.bfloat16)
    wprojT = np.ascontiguousarray(proj_w.T).astype(ml_dtypes.bfloat16)
    projb_f = proj_b + proj_w @ qkvb2[2 * DIM:]

    idx = np.arange(H)
    Rh = rel_pos_h[idx[:, None] - idx[None, :] + (H - 1)]  # (32,32,64) [q,k,c]
    Rw = rel_pos_w[idx[:, None] - idx[None, :] + (W - 1)]
    rhwT = np.concatenate(
        [Rh.transpose(2, 0, 1) / SCALE, Rw.transpose(2, 0, 1) / SCALE],
        axis=2)  # (64, 32, 64)
    rhwT = np.ascontiguousarray(rhwT).astype(ml_dtypes.bfloat16)
    zeros64 = np.zeros((64, H, 64), ml_dtypes.bfloat16)

    k = np.arange(N)
    kconst = np.zeros((64, N), np.float32)
    kconst[:32] = (k[None, :] // 32 == np.arange(32)[:, None])
    kconst[32:] = (k[None, :] % 32 == np.arange(32)[:, None])
    kconst = kconst.astype(ml_dtypes.bfloat16)

    ident = np.eye(128, dtype=ml_dtypes.bfloat16)

    shared = dict(wpack=wpack, qkvb=qkvb2, wprojT=wprojT, projb=projb_f,
                  rhwT=rhwT, zeros64=zeros64, kconst=kconst, ident=ident)
    in_maps = []
    for b in range(B):
        xTb = np.ascontiguousarray(x[b].reshape(N, DIM).T).astype(ml_dtypes.bfloat16)
        in_maps.append(dict(xT=xTb, **shared))
    return in_maps


def get_nc():
    if "nc" not in _CACHE:
        _CACHE["nc"] = build_nc()
    return _CACHE["nc"]


def kernel(**inputs):
    nc = get_nc()
    in_maps = host_prep(**inputs)
    res = bass_utils.run_bass_kernel_spmd(nc, in_maps, core_ids=list(range(NCORES)))
    out = np.stack([np.asarray(r["y"]).T for r in res.results], axis=0)
    return np.ascontiguousarray(out).reshape(B, H, W, DIM).astype(np.float32)


# revision 33
# speedup vs baseline: 1.2195x; 1.0499x over previous
"""Trainium2 Bass kernel: ViT-style global attention with decomposed
relative position bias (B=8, 32x32 tokens, dim 768, 12 heads, hd 64).

Sharding: data-parallel over batch B=8 -> one image per NeuronCore,
weights replicated, no collectives.

Fully software-pipelined single pass over 6 head-pairs:
  - production(P): q/k features (feature-major, fp32 PSUM chains over
    XT), V (token-major), rel-pos contraction per pair; emitted as PE
    "fillers" interleaved into the previous pair's attention so the PE
    never starves while ACT runs exp.
  - attention(head): per kb ONE K=128 matmul gives scale*S^T+bias in
    PSUM (bias rides contraction rows 64:128 vs indicator rows of K');
    exp on ACT -> PT (bf16). PV is token-major: lhsT = PT 128x128
    chunk, rhs = V[keys,65] (64 v-feats + ones col) -> out[tok,65]
    accumulated over kb; col 64 = softmax denominator for free.
  - cleanup(pair): reciprocal of D (DVE), divide via stride-0
    broadcast tensor_tensor (DVE), PE transpose back to feature-major
    AOD (bf16) for proj.
  - proj: fp32-accumulated bf16 matmuls + bias, y^T DMA'd out per
    chunk; host does the final transpose during unsharding.
Engine budget: PE ~135us (bound), ACT ~115us (exp), DVE ~60us,
Pool ~70us (drains).
"""

import os

import numpy as np

import concourse.bacc as bacc
import concourse.bass as bass
import concourse.tile as tile
from concourse import mybir
from concourse import bass_utils

B, H, W, DIM = 8, 32, 32, 768
HEADS, HD = 12, 64
N = H * W  # 1024
NCORES = 8
SCALE = HD ** -0.5
F32 = mybir.dt.float32
F32R = mybir.dt.float32r
BF16 = mybir.dt.bfloat16
EXP = mybir.ActivationFunctionType.Exp
IDN = mybir.ActivationFunctionType.Identity
ADD = mybir.AluOpType.add
MULT = mybir.AluOpType.mult

NC = DIM // 128      # 6 feature chunks == head pairs
NKB = N // 128       # 8 key/token blocks
NQH = N // 512       # 2 query halves

_CACHE = {}

NWARM = int(os.environ.get("K_WARM", "11"))
PT_BUFS = int(os.environ.get("K_PT", "16"))


def build_nc():
    nc = bacc.Bacc("TRN2", target_bir_lowering=False, debug=False)

    xT = nc.dram_tensor("xT", (DIM, N), BF16, kind="ExternalInput").ap()
    # packed per-pair weight columns: (pair, DIM, 384) = (q_p | k_p | v_p)
    wpack = nc.dram_tensor("wpack", (NC, DIM, 384), BF16, kind="ExternalInput").ap()
    qkvb = nc.dram_tensor("qkvb", (3 * DIM,), F32, kind="ExternalInput").ap()
    wprojT = nc.dram_tensor("wprojT", (DIM, DIM), BF16, kind="ExternalInput").ap()
    projb = nc.dram_tensor("projb", (DIM,), F32, kind="ExternalInput").ap()
    # rhwT: [64, H, 64] = (RhT | RwT) blocks; zeros64 fills the off-blocks
    rhwT = nc.dram_tensor("rhwT", (64, H, 64), BF16, kind="ExternalInput").ap()
    zeros64 = nc.dram_tensor("zeros64", (64, H, 64), BF16, kind="ExternalInput").ap()
    kconst = nc.dram_tensor("kconst", (64, N), BF16, kind="ExternalInput").ap()
    ident = nc.dram_tensor("ident", (128, 128), BF16, kind="ExternalInput").ap()
    y = nc.dram_tensor("y", (DIM, N), F32, kind="ExternalOutput").ap()

    qkvb2 = qkvb.rearrange("(c p one) -> c p one", p=128, one=1)   # [18][128,1]

    with tile.TileContext(nc) as tc:
        # ---- long-lived pools (bottom of SBUF stack) ----
        qall_p = tc.alloc_tile_pool(name="qall", bufs=1)
        kall_p = tc.alloc_tile_pool(name="kall", bufs=1)
        vall_p = tc.alloc_tile_pool(name="vall", bufs=1)
        cons_p = tc.alloc_tile_pool(name="cons", bufs=1)
        xt_p = tc.alloc_tile_pool(name="xtp", bufs=1)
        aod_p = tc.alloc_tile_pool(name="aod", bufs=1)

        QALL = qall_p.tile([128, HEADS, N], BF16)
        KALL = kall_p.tile([128, HEADS, N], BF16)
        VALL = vall_p.tile([128, HEADS, NKB, 65], BF16)
        STQB = cons_p.tile([128, NC, N], BF16)
        BD = cons_p.tile([128, H, 128], BF16)
        IDT = cons_p.tile([128, 128], BF16)
        PBIAS = cons_p.tile([128, NC], F32)
        XT = xt_p.tile([128, NC, N], BF16, tag="xtslot")
        AOD = aod_p.tile([128, NC, N], BF16)

        with tc.tile_pool(name="wpool", bufs=1) as w_p, \
             tc.tile_pool(name="bias", bufs=4) as b_p, \
             tc.tile_pool(name="pt", bufs=PT_BUFS) as pt_p, \
             tc.tile_pool(name="stg", bufs=2) as stg_p, \
             tc.tile_pool(name="psA", bufs=2, space="PSUM") as psA_p, \
             tc.tile_pool(name="psB", bufs=2, space="PSUM") as psB_p:

            # ---------- constant / early DMAs ----------
            wts = {}

            def dma_pair_weights(p):
                wt = w_p.tile([128, NC, 384], BF16, tag="wt", bufs=2,
                              name=f"wt{p}")
                nc.sync.dma_start(
                    out=wt, in_=wpack[p].rearrange("(c p2) f -> p2 c f", p2=128))
                wts[p] = wt

            def dma_pair_biases(p):
                qb = b_p.tile([128, 1], F32, tag="bias", bufs=6, name=f"qb{p}")
                nc.sync.dma_start(out=qb, in_=qkvb2[p])
                kb2 = b_p.tile([128, 1], F32, tag="bias", bufs=6, name=f"kb{p}")
                nc.sync.dma_start(out=kb2, in_=qkvb2[6 + p])
                return qb, kb2

            def dma_pair_kconst(p):
                nc.sync.dma_start(out=KALL[64:128, 2 * p, :], in_=kconst)
                nc.sync.dma_start(out=KALL[0:64, 2 * p + 1, :], in_=kconst)

            # PE p-state warm-up while DMAs land (memset first so DVE's later
            # big memsets don't gate the first warm matmul)
            if NWARM:
                jnk = cons_p.tile([128, 512], BF16)
                nc.vector.memset(jnk, 0.5)
                wps = psA_p.tile([128, 512], F32, tag="ps1", name="warmps")
                for _ in range(NWARM):
                    nc.tensor.matmul(wps, lhsT=jnk[:, 0:128], rhs=jnk,
                                     start=True, stop=True,
                                     skip_group_check=True)

            dma_pair_weights(0)
            dma_pair_kconst(0)
            xTr = xT.rearrange("(c p2) n -> p2 c n", p2=128)
            for qh in range(NQH):
                qsl = slice(qh * 512, (qh + 1) * 512)
                nc.sync.dma_start(out=XT[:, :, qsl], in_=xTr[:, :, qsl])
            biases = {0: dma_pair_biases(0)}
            dma_pair_weights(1)
            biases[1] = dma_pair_biases(1)
            dma_pair_kconst(1)
            # block-diagonal rel stationary: the four placements occupy
            # disjoint column ranges, so rel-h and rel-w share one tile (the
            # cross terms land in output rows that are never copied out).
            # Fully DMA'd (incl. zero blocks) to keep DVE off the lead-in.
            nc.sync.dma_start(out=BD[0:64, :, 64:128], in_=rhwT)
            nc.sync.dma_start(out=BD[64:128, :, 0:64], in_=rhwT)
            nc.sync.dma_start(out=BD[0:64, :, 0:64], in_=zeros64)
            nc.sync.dma_start(out=BD[64:128, :, 64:128], in_=zeros64)
            nc.sync.dma_start(out=IDT, in_=ident)
            nc.vector.memset(VALL.rearrange("p h k c -> p (h k) c")[:, :, 64:65], 1.0)

            # ---------- production of one pair (generator of PE pieces) ----
            stq4 = STQB.rearrange("p c (t ww) -> p c t ww", ww=W)

            def gen_production(p, act_drains=False):
                """Yield rows_emitted after each small PE piece. With
                act_drains (prologue only, ACT otherwise idle) the even-half
                drains go to ACT to halve the lead-in drain latency."""
                wt = wts[p]
                qb, kb2 = biases[p]
                he, ho = 2 * p, 2 * p + 1

                def drain_even(out, in0, sc):
                    if act_drains:
                        nc.scalar.activation(out, in0, IDN, bias=sc, scale=1.0)
                    else:
                        nc.vector.tensor_scalar(out=out, in0=in0, scalar1=sc,
                                                scalar2=None, op0=ADD)

                def copy_even(out, in0):
                    if act_drains:
                        nc.scalar.copy(out, in0)
                    else:
                        nc.vector.tensor_copy(out, in0)
                # q chains (feature-major)
                for qh in range(NQH):
                    qsl = slice(qh * 512, (qh + 1) * 512)
                    ps = psA_p.tile([128, 512], F32, tag="ps1",
                                    name=f"psq{p}_{qh}")
                    for c in range(NC):
                        nc.tensor.matmul(ps, lhsT=wt[:, c, 0:128],
                                         rhs=XT[:, c, qsl],
                                         start=(c == 0), stop=(c == NC - 1))
                        yield 512
                    drain_even(QALL[0:64, he, qsl], ps[0:64], qb[0:64])
                    nc.vector.tensor_scalar(out=QALL[64:128, ho, qsl],
                                            in0=ps[64:128], scalar1=qb[64:128],
                                            scalar2=None, op0=ADD)
                    nc.vector.tensor_scalar(out=STQB[:, p, qsl], in0=ps,
                                            scalar1=qb, scalar2=None, op0=ADD)
                    yield 0
                # k chains
                for qh in range(NQH):
                    qsl = slice(qh * 512, (qh + 1) * 512)
                    ps = psA_p.tile([128, 512], F32, tag="ps1",
                                    name=f"psk{p}_{qh}")
                    for c in range(NC):
                        nc.tensor.matmul(ps, lhsT=wt[:, c, 128:256],
                                         rhs=XT[:, c, qsl],
                                         start=(c == 0), stop=(c == NC - 1))
                        yield 512
                    drain_even(KALL[0:64, he, qsl], ps[0:64], kb2[0:64])
                    nc.vector.tensor_scalar(out=KALL[64:128, ho, qsl],
                                            in0=ps[64:128], scalar1=kb2[64:128],
                                            scalar2=None, op0=ADD)
                    yield 0
                # rel-pos: h then w, in two 16-row halves
                for half in range(2):
                    hsl = slice(half * 512, (half + 1) * 512)
                    psr = psA_p.tile([128, 16, 32], F32, tag="ps1",
                                     name=f"psrh{p}_{half}")
                    for j in range(16):
                        h = half * 16 + j
                        nc.tensor.matmul(psr[:, j, :], lhsT=BD[:, h, :],
                                         rhs=STQB[:, p, h * 32:(h + 1) * 32],
                                         start=True, stop=True,
                                         skip_group_check=True)
                        if j % 4 == 3:
                            yield 128
                    copy_even(
                        QALL[64:96, he, hsl].rearrange("p (hb t) -> p hb t", t=32),
                        psr[64:96])
                    nc.vector.tensor_copy(
                        QALL[0:32, ho, hsl].rearrange("p (hb t) -> p hb t", t=32),
                        psr[0:32])
                    yield 0
                qvwE = QALL[96:128, he, :].rearrange("p (t wb) -> p wb t", wb=32)
                qvwO = QALL[32:64, ho, :].rearrange("p (t wb) -> p wb t", wb=32)
                for half in range(2):
                    psr = psA_p.tile([128, 16, 32], F32, tag="ps1",
                                     name=f"psrw{p}_{half}")
                    for j in range(16):
                        w = half * 16 + j
                        nc.tensor.matmul(psr[:, j, :], lhsT=BD[:, w, :],
                                         rhs=stq4[:, p, :, w],
                                         start=True, stop=True,
                                         skip_group_check=True)
                        if j % 4 == 3:
                            yield 128
                    copy_even(qvwE[:, half * 16:(half + 1) * 16, :],
                              psr[96:128])
                    nc.vector.tensor_copy(qvwO[:, half * 16:(half + 1) * 16, :],
                                          psr[32:64])
                    yield 0

                # v chains (token-major), 2 token-blocks per psum tile,
                # single ACT drain per tile. v-bias is folded into the proj
                # bias on the host (y is affine in v).
                for vb in range(4):
                    psv = psA_p.tile([128, 2, 128], F32, tag="ps1",
                                     name=f"psv{p}_{vb}")
                    for t2 in range(2):
                        tb = vb * 2 + t2
                        for c in range(NC):
                            nc.tensor.matmul(
                                psv[:, t2, :],
                                lhsT=XT[:, c, tb * 128:(tb + 1) * 128],
                                rhs=wt[:, c, 256:384],
                                start=(c == 0), stop=(c == NC - 1),
                                skip_group_check=True)
                            yield 128
                    if act_drains:
                        nc.scalar.copy(
                            VALL[:, he:ho + 1, vb * 2:(vb + 1) * 2, 0:64],
                            psv.rearrange("p t (hh f) -> p hh t f", hh=2))
                    else:
                        nc.vector.tensor_copy(
                            VALL[:, he:ho + 1, vb * 2:(vb + 1) * 2, 0:64],
                            psv.rearrange("p t (hh f) -> p hh t f", hh=2))
                    yield 0
            # ---------- attention pieces ----------
            def emit_qk_exp(h, kb):
                ps_s = psA_p.tile([128, N], F32, tag="s", name=f"s{h}_{kb}")
                for qh in range(NQH):
                    nc.tensor.matmul(
                        ps_s[:, qh * 512:(qh + 1) * 512],
                        lhsT=KALL[:, h, kb * 128:(kb + 1) * 128],
                        rhs=QALL[:, h, qh * 512:(qh + 1) * 512],
                        start=True, stop=True)
                pt = pt_p.tile([128, N], BF16, tag="pt", name=f"pt{h}_{kb}")
                nc.scalar.activation(pt, ps_s, EXP)
                return pt

            def emit_pv(h, pts, aodt):
                """PV in two 4-token-block halves; the softmax divide is
                fused into the drain: one reciprocal per half + one
                stride-0-broadcast multiply PSUM->aodt."""
                par = h % 2
                fsl = slice(par * 64, par * 64 + 64)
                for half in range(2):
                    pv = psB_p.tile([128, 4, 128], F32, tag="pv",
                                    name=f"pv{h}_{half}")
                    for t2 in range(4):
                        tb = half * 4 + t2
                        for kb in range(NKB):
                            nc.tensor.matmul(
                                pv[:, t2, 0:65],
                                lhsT=pts[kb][:, tb * 128:(tb + 1) * 128],
                                rhs=VALL[:, h, kb, :],
                                start=(kb == 0), stop=(kb == NKB - 1),
                                skip_group_check=True)
                    rh = stg_p.tile([128, 4], F32, tag="rd", bufs=4,
                                    name=f"rd{h}_{half}")
                    nc.vector.reciprocal(
                        rh, pv[:, :, 64:65].rearrange("p t one -> p (t one)"))
                    in1 = bass.AP(tensor=rh.tensor, offset=rh.offset,
                                  ap=[list(rh.ap[0]), [1, 4], [0, 64]])
                    nc.vector.tensor_mul(
                        aodt[:, half * 4:(half + 1) * 4, fsl],
                        pv[:, :, 0:64], in1)

            def emit_cleanup(p, aodt):
                for tp in range(4):
                    pst = psA_p.tile([128, 2, 128], BF16, tag="ps1",
                                     name=f"pst{p}_{tp}")
                    for t2 in range(2):
                        nc.tensor.transpose(pst[:, t2, :],
                                            aodt[:, tp * 2 + t2, :], IDT)
                    nc.vector.tensor_copy(
                        AOD[:, p, tp * 256:(tp + 1) * 256],
                        pst.rearrange("p a b -> p (a b)"))

            # ---------- the pipelined main loop ----------
            fillers = None      # generator producing pair p+1
            prev_pts = None     # PT tiles of previous head
            prev_aodt = None    # token-major attention-out of prev head's pair

            # prologue: produce pair 0 outright, then bridge the drain
            # window (QK(0,0) waits on DVE/ACT drains) with early pulls of
            # production(1) so the PE never idles nor drops its p-state.
            for _ in gen_production(0, act_drains=True):
                pass
            fillers = gen_production(1)
            bridge = 6144.0
            while fillers is not None and bridge > 0:
                try:
                    bridge -= next(fillers)
                except StopIteration:
                    fillers = None

            PAIR_ROWS = 20480.0  # PE rows per pair production

            aodts = {}
            for j in range(HEADS):
                h = j
                p = j // 2
                par = j % 2
                if par == 0:
                    aodts[p] = stg_p.tile([128, NKB, 128], BF16, tag="aodt",
                                          name=f"aodt{p}")
                    # DMAs for pair p+2 production (consumed via fillers at
                    # steps 2p+2, 2p+3)
                    if p + 2 < NC:
                        dma_pair_weights(p + 2)
                        biases[p + 2] = dma_pair_biases(p + 2)
                        dma_pair_kconst(p + 2)
                    if p == 2:
                        WP = cons_p.tile([128, NC, DIM], BF16)
                        for c in range(NC):
                            nc.sync.dma_start(
                                out=WP[:, c, :],
                                in_=wprojT[c * 128:(c + 1) * 128, :])
                        nc.sync.dma_start(
                            out=PBIAS, in_=projb.rearrange("(c p) -> p c", p=128))
                    # production of pair p+1 interleaves into this pair's steps
                    if fillers is None and p + 1 < NC:
                        fillers = gen_production(p + 1)

                # QK + exp for head h, pulling fillers to keep PE fed.
                # Front-load q/k/rel of the next pair into the even head's
                # slots so the next pair's QK is never production-gated;
                # only v (needed a step later) rides the odd head's slots.
                pts = []
                budget = 0.0
                for kb in range(NKB):
                    pts.append(emit_qk_exp(h, kb))
                    budget += (14336.0 if par == 0 else 6144.0) / 8.0
                    while fillers is not None and budget > 0:
                        try:
                            budget -= next(fillers)
                        except StopIteration:
                            fillers = None
                # cleanup of pair p-1: its last PV was emitted in the
                # previous step; deferring to after this step's QK loop gives
                # the DVE recip/divide chain a full QK window to complete
                # before the PE reaches the transposes.
                if par == 1 and p >= 1:
                    emit_cleanup(p - 1, aodts.pop(p - 1))

                # PV of the previous head
                if prev_pts is not None:
                    emit_pv(h - 1, prev_pts, prev_aodt)
                prev_pts, prev_aodt = pts, aodts[p]

                # drain any residual production at pair boundaries
                if par == 1 and fillers is not None:
                    for _ in fillers:
                        pass
                    fillers = None

            emit_pv(HEADS - 1, prev_pts, prev_aodt)
            emit_cleanup(NC - 1, aodts.pop(NC - 1))

        # ---------- proj + bias + out ----------
        with tc.tile_pool(name="ps4", bufs=6, space="PSUM") as ps4_p, \
             tc.tile_pool(name="wpp", bufs=1) as wp2_p:
            YSB = xt_p.tile([128, NC, N], F32, tag="xtslot")
            for ob in range(NC):
                for qh in range(NQH):
                    qsl = slice(qh * 512, (qh + 1) * 512)
                    ps = ps4_p.tile([128, 512], F32, tag="ps4",
                                    name=f"psp{ob}_{qh}")
                    for c in range(NC):
                        nc.tensor.matmul(
                            ps, lhsT=WP[:, c, ob * 128:(ob + 1) * 128],
                            rhs=AOD[:, c, qsl],
                            start=(c == 0), stop=(c == NC - 1))
                    nc.vector.tensor_scalar(
                        out=YSB[:, ob, qsl], in0=ps,
                        scalar1=PBIAS[:, ob:ob + 1], scalar2=None, op0=ADD)
                    nc.sync.dma_start(out=y[ob * 128:(ob + 1) * 128, qsl],
                                      in_=YSB[:, ob, qsl])
        aod_p.release()
        xt_p.release()
        cons_p.release()
        vall_p.release()
        kall_p.release()
        qall_p.release()

    nc.compile()
    return nc


def host_prep(x, qkv_w, qkv_b, proj_w, proj_b, rel_pos_h, rel_pos_w):
    """full inputs -> list of 8 per-core in_maps"""
    import ml_dtypes
    x = np.asarray(x, np.float32)
    qkv_w = np.asarray(qkv_w, np.float32)
    qkv_b = np.asarray(qkv_b, np.float32)
    proj_w = np.asarray(proj_w, np.float32)
    proj_b = np.asarray(proj_b, np.float32)
    rel_pos_h = np.asarray(rel_pos_h, np.float32)
    rel_pos_w = np.asarray(rel_pos_w, np.float32)

    wqkvT = np.ascontiguousarray(qkv_w.T).copy()   # (768, 2304)
    wqkvT[:, :DIM] *= SCALE
    qkvb2 = qkv_b.copy()
    qkvb2[:DIM] *= SCALE
    # packed (pair, 768, 384) = q_p | k_p | v_p
    wpack = np.empty((NC, DIM, 384), np.float32)
    for p in range(NC):
        wpack[p, :, 0:128] = wqkvT[:, p * 128:(p + 1) * 128]
        wpack[p, :, 128:256] = wqkvT[:, DIM + p * 128:DIM + (p + 1) * 128]
        wpack[p, :, 256:384] = wqkvT[:, 2 * DIM + p * 128:2 * DIM + (p + 1) * 128]
    wpack = wpack.astype(ml_dtypes.bfloat16)
    wprojT = np.ascontiguousarray(proj_w.T).astype(ml_dtypes.bfloat16)
    projb_f = proj_b + proj_w @ qkvb2[2 * DIM:]

    idx = np.arange(H)
    Rh = rel_pos_h[idx[:, None] - idx[None, :] + (H - 1)]  # (32,32,64) [q,k,c]
    Rw = rel_pos_w[idx[:, None] - idx[None, :] + (W - 1)]
    rhwT = np.concatenate(
        [Rh.transpose(2, 0, 1) / SCALE, Rw.transpose(2, 0, 1) / SCALE],
        axis=2)  # (64, 32, 64)
    rhwT = np.ascontiguousarray(rhwT).astype(ml_dtypes.bfloat16)
    zeros64 = np.zeros((64, H, 64), ml_dtypes.bfloat16)

    k = np.arange(N)
    kconst = np.zeros((64, N), np.float32)
    kconst[:32] = (k[None, :] // 32 == np.arange(32)[:, None])
    kconst[32:] = (k[None, :] % 32 == np.arange(32)[:, None])
    kconst = kconst.astype(ml_dtypes.bfloat16)

    ident = np.eye(128, dtype=ml_dtypes.bfloat16)

    shared = dict(wpack=wpack, qkvb=qkvb2, wprojT=wprojT, projb=projb_f,
                  rhwT=rhwT, zeros64=zeros64, kconst=kconst, ident=ident)
    in_maps = []
    for b in range(B):
        xTb = np.ascontiguousarray(x[b].reshape(N, DIM).T).astype(ml_dtypes.bfloat16)
        in_maps.append(dict(xT=xTb, **shared))
    return in_maps


def get_nc():
    if "nc" not in _CACHE:
        _CACHE["nc"] = build_nc()
    return _CACHE["nc"]


def kernel(**inputs):
    nc = get_nc()
    in_maps = host_prep(**inputs)
    res = bass_utils.run_bass_kernel_spmd(nc, in_maps, core_ids=list(range(NCORES)))
    out = np.stack([np.asarray(r["y"]).T for r in res.results], axis=0)
    return np.ascontiguousarray(out).reshape(B, H, W, DIM).astype(np.float32)


# revision 44
# speedup vs baseline: 1.2730x; 1.0439x over previous
"""Trainium2 Bass kernel: ViT-style global attention with decomposed
relative position bias (B=8, 32x32 tokens, dim 768, 12 heads, hd 64).

Sharding: data-parallel over batch B=8 -> one image per NeuronCore,
weights replicated, no collectives.

Fully software-pipelined single pass over 6 head-pairs:
  - production(P): q/k features (feature-major, fp32 PSUM chains over
    XT), V (token-major), rel-pos contraction per pair; emitted as PE
    "fillers" interleaved into the previous pair's attention so the PE
    never starves while ACT runs exp.
  - attention(head): per kb ONE K=128 matmul gives scale*S^T+bias in
    PSUM (bias rides contraction rows 64:128 vs indicator rows of K');
    exp on ACT -> PT (bf16). PV is token-major: lhsT = PT 128x128
    chunk, rhs = V[keys,65] (64 v-feats + ones col) -> out[tok,65]
    accumulated over kb; col 64 = softmax denominator for free.
  - cleanup(pair): reciprocal of D (DVE), divide via stride-0
    broadcast tensor_tensor (DVE), PE transpose back to feature-major
    AOD (bf16) for proj.
  - proj: fp32-accumulated bf16 matmuls + bias, y^T DMA'd out per
    chunk; host does the final transpose during unsharding.
Engine budget: PE ~135us (bound), ACT ~115us (exp), DVE ~60us,
Pool ~70us (drains).
"""

import os

import numpy as np

import concourse.bacc as bacc
import concourse.bass as bass
import concourse.tile as tile
from concourse import mybir
from concourse import bass_utils

B, H, W, DIM = 8, 32, 32, 768
HEADS, HD = 12, 64
N = H * W  # 1024
NCORES = 8
SCALE = HD ** -0.5
F32 = mybir.dt.float32
F32R = mybir.dt.float32r
BF16 = mybir.dt.bfloat16
EXP = mybir.ActivationFunctionType.Exp
IDN = mybir.ActivationFunctionType.Identity
ADD = mybir.AluOpType.add
MULT = mybir.AluOpType.mult

NC = DIM // 128      # 6 feature chunks == head pairs
NKB = N // 128       # 8 key/token blocks
NQH = N // 512       # 2 query halves

_CACHE = {}

NWARM = int(os.environ.get("K_WARM", "8"))
PT_BUFS = int(os.environ.get("K_PT", "16"))


def build_nc():
    nc = bacc.Bacc("TRN2", target_bir_lowering=False, debug=False)

    xT = nc.dram_tensor("xT", (DIM, N), BF16, kind="ExternalInput").ap()
    # packed per-pair weight columns: (pair, DIM, 384) = (q_p | k_p | v_p)
    wpack = nc.dram_tensor("wpack", (NC, DIM, 384), BF16, kind="ExternalInput").ap()
    qkvb = nc.dram_tensor("qkvb", (3 * DIM,), F32, kind="ExternalInput").ap()
    wprojT = nc.dram_tensor("wprojT", (DIM, DIM), BF16, kind="ExternalInput").ap()
    projb = nc.dram_tensor("projb", (DIM,), F32, kind="ExternalInput").ap()
    # bdfull: host-packed block-diagonal rel stationary [128, H, 128]
    bdfull = nc.dram_tensor("bdfull", (128, H, 128), BF16, kind="ExternalInput").ap()
    kconst = nc.dram_tensor("kconst", (64, N), BF16, kind="ExternalInput").ap()
    ident = nc.dram_tensor("ident", (128, 128), BF16, kind="ExternalInput").ap()
    y = nc.dram_tensor("y", (DIM, N), F32, kind="ExternalOutput").ap()

    qkvb2 = qkvb.rearrange("(c p one) -> c p one", p=128, one=1)   # [18][128,1]

    with tile.TileContext(nc) as tc:
        # ---- long-lived pools (bottom of SBUF stack) ----
        qall_p = tc.alloc_tile_pool(name="qall", bufs=1)
        kall_p = tc.alloc_tile_pool(name="kall", bufs=1)
        vall_p = tc.alloc_tile_pool(name="vall", bufs=1)
        cons_p = tc.alloc_tile_pool(name="cons", bufs=1)
        xt_p = tc.alloc_tile_pool(name="xtp", bufs=1)
        aod_p = tc.alloc_tile_pool(name="aod", bufs=1)

        QALL = qall_p.tile([128, HEADS, N], BF16)
        KALL = kall_p.tile([128, HEADS, N], BF16)
        VALL = vall_p.tile([128, HEADS, NKB, 65], BF16)
        STQB = cons_p.tile([128, NC, N], BF16)
        BD = cons_p.tile([128, H, 128], BF16)
        IDT = cons_p.tile([128, 128], BF16)
        PBIAS = cons_p.tile([128, NC], F32)
        YA = cons_p.tile([128, NC, N], F32)   # proj partial (c=0..2)
        XT = xt_p.tile([128, NC, N], BF16, tag="xtslot")
        AOD = aod_p.tile([128, NC, N], BF16)

        with tc.tile_pool(name="wpool", bufs=1) as w_p, \
             tc.tile_pool(name="bias", bufs=4) as b_p, \
             tc.tile_pool(name="pt", bufs=PT_BUFS) as pt_p, \
             tc.tile_pool(name="stg", bufs=2) as stg_p, \
             tc.tile_pool(name="psA", bufs=2, space="PSUM") as psA_p, \
             tc.tile_pool(name="psB", bufs=2, space="PSUM") as psB_p:

            # ---------- constant / early DMAs ----------
            wts = {}

            def dma_pair_weights(p):
                wt = w_p.tile([128, NC, 384], BF16, tag="wt", bufs=2,
                              name=f"wt{p}")
                nc.sync.dma_start(
                    out=wt, in_=wpack[p].rearrange("(c p2) f -> p2 c f", p2=128))
                wts[p] = wt

            def dma_pair_biases(p):
                qb = b_p.tile([128, 1], F32, tag="bias", bufs=6, name=f"qb{p}")
                nc.sync.dma_start(out=qb, in_=qkvb2[p])
                kb2 = b_p.tile([128, 1], F32, tag="bias", bufs=6, name=f"kb{p}")
                nc.sync.dma_start(out=kb2, in_=qkvb2[6 + p])
                return qb, kb2

            def dma_pair_kconst(p):
                nc.sync.dma_start(out=KALL[64:128, 2 * p, :], in_=kconst)
                nc.sync.dma_start(out=KALL[0:64, 2 * p + 1, :], in_=kconst)

            # PE p-state warm-up while DMAs land (memset first so DVE's later
            # big memsets don't gate the first warm matmul)
            if NWARM:
                jnk = cons_p.tile([128, 512], BF16)
                nc.vector.memset(jnk, 0.5)
                # preload the Exp table while the PE warms up so the first
                # real exp doesn't eat the 1.3us table load
                nc.scalar.activation(jnk[:, 0:8], jnk[:, 0:8], EXP)
                wps = psA_p.tile([128, 512], F32, tag="ps1", name="warmps")
                for _ in range(NWARM):
                    nc.tensor.matmul(wps, lhsT=jnk[:, 0:128], rhs=jnk,
                                     start=True, stop=True,
                                     skip_group_check=True)

            dma_pair_weights(0)
            biases = {0: dma_pair_biases(0)}
            biases[1] = dma_pair_biases(1)
            xTr = xT.rearrange("(c p2) n -> p2 c n", p2=128)
            for c in range(NC):
                nc.sync.dma_start(out=XT[:, c, :], in_=xTr[:, c, :])
            nc.sync.dma_start(out=BD, in_=bdfull)
            dma_pair_kconst(0)
            dma_pair_weights(1)
            dma_pair_kconst(1)
            nc.sync.dma_start(out=IDT, in_=ident)
            nc.vector.memset(VALL.rearrange("p h k c -> p (h k) c")[:, :, 64:65], 1.0)

            # ---------- production of one pair (generator of PE pieces) ----
            stq4 = STQB.rearrange("p c (t ww) -> p c t ww", ww=W)

            def gen_production(p, act_drains=False):
                """Yield rows_emitted after each small PE piece. With
                act_drains (prologue only, ACT otherwise idle) the even-half
                drains go to ACT to halve the lead-in drain latency."""
                wt = wts[p]
                qb, kb2 = biases[p]
                he, ho = 2 * p, 2 * p + 1

                def drain_even(out, in0, sc):
                    if act_drains:
                        nc.scalar.activation(out, in0, IDN, bias=sc, scale=1.0)
                    else:
                        nc.vector.tensor_scalar(out=out, in0=in0, scalar1=sc,
                                                scalar2=None, op0=ADD)

                def copy_even(out, in0):
                    if act_drains:
                        nc.scalar.copy(out, in0)
                    else:
                        nc.vector.tensor_copy(out, in0)
                # q chains (feature-major)
                for qh in range(NQH):
                    qsl = slice(qh * 512, (qh + 1) * 512)
                    ps = psA_p.tile([128, 512], F32, tag="ps1",
                                    name=f"psq{p}_{qh}")
                    for c in range(NC):
                        nc.tensor.matmul(ps, lhsT=wt[:, c, 0:128],
                                         rhs=XT[:, c, qsl],
                                         start=(c == 0), stop=(c == NC - 1))
                        yield 512
                    drain_even(QALL[0:64, he, qsl], ps[0:64], qb[0:64])
                    nc.vector.tensor_scalar(out=QALL[64:128, ho, qsl],
                                            in0=ps[64:128], scalar1=qb[64:128],
                                            scalar2=None, op0=ADD)
                    if act_drains:
                        nc.scalar.activation(STQB[:, p, qsl], ps, IDN,
                                             bias=qb, scale=1.0)
                    else:
                        nc.vector.tensor_scalar(out=STQB[:, p, qsl], in0=ps,
                                                scalar1=qb, scalar2=None,
                                                op0=ADD)
                    yield 0
                # k chains
                for qh in range(NQH):
                    qsl = slice(qh * 512, (qh + 1) * 512)
                    ps = psA_p.tile([128, 512], F32, tag="ps1",
                                    name=f"psk{p}_{qh}")
                    for c in range(NC):
                        nc.tensor.matmul(ps, lhsT=wt[:, c, 128:256],
                                         rhs=XT[:, c, qsl],
                                         start=(c == 0), stop=(c == NC - 1))
                        yield 512
                    drain_even(KALL[0:64, he, qsl], ps[0:64], kb2[0:64])
                    nc.vector.tensor_scalar(out=KALL[64:128, ho, qsl],
                                            in0=ps[64:128], scalar1=kb2[64:128],
                                            scalar2=None, op0=ADD)
                    yield 0
                # rel-pos: h then w, in two 16-row halves
                for half in range(2):
                    hsl = slice(half * 512, (half + 1) * 512)
                    psr = psA_p.tile([128, 16, 32], F32, tag="ps1",
                                     name=f"psrh{p}_{half}")
                    for j in range(16):
                        h = half * 16 + j
                        nc.tensor.matmul(psr[:, j, :], lhsT=BD[:, h, :],
                                         rhs=STQB[:, p, h * 32:(h + 1) * 32],
                                         start=True, stop=True,
                                         skip_group_check=True)
                        if j % 4 == 3:
                            yield 128
                    copy_even(
                        QALL[64:96, he, hsl].rearrange("p (hb t) -> p hb t", t=32),
                        psr[64:96])
                    nc.vector.tensor_copy(
                        QALL[0:32, ho, hsl].rearrange("p (hb t) -> p hb t", t=32),
                        psr[0:32])
                    yield 0
                qvwE = QALL[96:128, he, :].rearrange("p (t wb) -> p wb t", wb=32)
                qvwO = QALL[32:64, ho, :].rearrange("p (t wb) -> p wb t", wb=32)
                for half in range(2):
                    psr = psA_p.tile([128, 16, 32], F32, tag="ps1",
                                     name=f"psrw{p}_{half}")
                    for j in range(16):
                        w = half * 16 + j
                        nc.tensor.matmul(psr[:, j, :], lhsT=BD[:, w, :],
                                         rhs=stq4[:, p, :, w],
                                         start=True, stop=True,
                                         skip_group_check=True)
                        if j % 4 == 3:
                            yield 128
                    copy_even(qvwE[:, half * 16:(half + 1) * 16, :],
                              psr[96:128])
                    nc.vector.tensor_copy(qvwO[:, half * 16:(half + 1) * 16, :],
                                          psr[32:64])
                    yield 0

                # v chains (token-major), 2 token-blocks per psum tile,
                # single ACT drain per tile. v-bias is folded into the proj
                # bias on the host (y is affine in v).
                for vb in range(4):
                    psv = psA_p.tile([128, 2, 128], F32, tag="ps1",
                                     name=f"psv{p}_{vb}")
                    for t2 in range(2):
                        tb = vb * 2 + t2
                        for c in range(NC):
                            nc.tensor.matmul(
                                psv[:, t2, :],
                                lhsT=XT[:, c, tb * 128:(tb + 1) * 128],
                                rhs=wt[:, c, 256:384],
                                start=(c == 0), stop=(c == NC - 1),
                                skip_group_check=True)
                            yield 128
                    if act_drains:
                        nc.scalar.copy(
                            VALL[:, he:ho + 1, vb * 2:(vb + 1) * 2, 0:64],
                            psv.rearrange("p t (hh f) -> p hh t f", hh=2))
                    else:
                        nc.vector.tensor_copy(
                            VALL[:, he:ho + 1, vb * 2:(vb + 1) * 2, 0:64],
                            psv.rearrange("p t (hh f) -> p hh t f", hh=2))
                    yield 0
            # ---------- attention pieces ----------
            def emit_qk_exp(h, kb):
                ps_s = psA_p.tile([128, N], F32, tag="s", name=f"s{h}_{kb}")
                for qh in range(NQH):
                    nc.tensor.matmul(
                        ps_s[:, qh * 512:(qh + 1) * 512],
                        lhsT=KALL[:, h, kb * 128:(kb + 1) * 128],
                        rhs=QALL[:, h, qh * 512:(qh + 1) * 512],
                        start=True, stop=True)
                pt = pt_p.tile([128, N], BF16, tag="pt", name=f"pt{h}_{kb}")
                nc.scalar.activation(pt, ps_s, EXP)
                return pt

            def emit_pv(h, pts, aodt):
                """PV in two 4-token-block halves; the softmax divide is
                fused into the drain: one reciprocal per half + one
                stride-0-broadcast multiply PSUM->aodt."""
                par = h % 2
                fsl = slice(par * 64, par * 64 + 64)
                for half in range(2):
                    pv = psB_p.tile([128, 4, 128], F32, tag="pv",
                                    name=f"pv{h}_{half}")
                    for t2 in range(4):
                        tb = half * 4 + t2
                        for kb in range(NKB):
                            nc.tensor.matmul(
                                pv[:, t2, 0:65],
                                lhsT=pts[kb][:, tb * 128:(tb + 1) * 128],
                                rhs=VALL[:, h, kb, :],
                                start=(kb == 0), stop=(kb == NKB - 1),
                                skip_group_check=True)
                    rh = stg_p.tile([128, 4], F32, tag="rd", bufs=4,
                                    name=f"rd{h}_{half}")
                    nc.vector.reciprocal(
                        rh, pv[:, :, 64:65].rearrange("p t one -> p (t one)"))
                    in1 = bass.AP(tensor=rh.tensor, offset=rh.offset,
                                  ap=[list(rh.ap[0]), [1, 4], [0, 64]])
                    nc.vector.tensor_mul(
                        aodt[:, half * 4:(half + 1) * 4, fsl],
                        pv[:, :, 0:64], in1)

            def emit_cleanup(p, aodt):
                for tp in range(4):
                    pst = psA_p.tile([128, 2, 128], BF16, tag="ps1",
                                     name=f"pst{p}_{tp}")
                    for t2 in range(2):
                        nc.tensor.transpose(pst[:, t2, :],
                                            aodt[:, tp * 2 + t2, :], IDT)
                    nc.vector.tensor_copy(
                        AOD[:, p, tp * 256:(tp + 1) * 256],
                        pst.rearrange("p a b -> p (a b)"))

            # ---------- the pipelined main loop ----------
            fillers = None      # generator producing pair p+1
            prev_pts = None     # PT tiles of previous head
            prev_aodt = None    # token-major attention-out of prev head's pair

            # prologue: produce pair 0 outright, then bridge the drain
            # window (QK(0,0) waits on DVE/ACT drains) with early pulls of
            # production(1) so the PE never idles nor drops its p-state.
            for _ in gen_production(0, act_drains=True):
                pass
            fillers = gen_production(1)
            bridge = 6144.0
            while fillers is not None and bridge > 0:
                try:
                    bridge -= next(fillers)
                except StopIteration:
                    fillers = None

            PAIR_ROWS = 20480.0  # PE rows per pair production

            aodts = {}
            for j in range(HEADS):
                h = j
                p = j // 2
                par = j % 2
                if par == 0:
                    aodts[p] = stg_p.tile([128, NKB, 128], BF16, tag="aodt",
                                          name=f"aodt{p}")
                    # DMAs for pair p+2 production (consumed via fillers at
                    # steps 2p+2, 2p+3)
                    if p + 2 < NC:
                        dma_pair_weights(p + 2)
                        biases[p + 2] = dma_pair_biases(p + 2)
                        dma_pair_kconst(p + 2)
                    if p == 2:
                        WP = cons_p.tile([128, NC, DIM], BF16)
                        for c in range(NC):
                            nc.sync.dma_start(
                                out=WP[:, c, :],
                                in_=wprojT[c * 128:(c + 1) * 128, :])
                        nc.sync.dma_start(
                            out=PBIAS, in_=projb.rearrange("(c p) -> p c", p=128))
                    # production of pair p+1 interleaves into this pair's steps
                    if fillers is None and p + 1 < NC:
                        fillers = gen_production(p + 1)

                # QK + exp for head h, pulling fillers to keep PE fed.
                # Front-load q/k/rel of the next pair into the even head's
                # slots so the next pair's QK is never production-gated;
                # only v (needed a step later) rides the odd head's slots.
                pts = []
                budget = 0.0
                for kb in range(NKB):
                    pts.append(emit_qk_exp(h, kb))
                    budget += (14336.0 if par == 0 else 6144.0) / 8.0
                    while fillers is not None and budget > 0:
                        try:
                            budget -= next(fillers)
                        except StopIteration:
                            fillers = None
                # cleanup of pair p-1: its last PV was emitted in the
                # previous step; deferring to after this step's QK loop gives
                # the DVE recip/divide chain a full QK window to complete
                # before the PE reaches the transposes.
                if par == 1 and p >= 1:
                    emit_cleanup(p - 1, aodts.pop(p - 1))

                # proj first-half (c=0..2 over AOD pairs 0-2, ready after
                # cleanup(2)) as late-step fillers where production runs dry
                if j >= 9 and j <= 11:
                    for ob4 in range(4):
                        ob, qh = divmod((j - 9) * 4 + ob4, NQH)
                        qsl = slice(qh * 512, (qh + 1) * 512)
                        ps = psA_p.tile([128, 512], F32, tag="ps1",
                                        name=f"psyA{ob}_{qh}")
                        for c in range(4):
                            nc.tensor.matmul(
                                ps, lhsT=WP[:, c, ob * 128:(ob + 1) * 128],
                                rhs=AOD[:, c, qsl],
                                start=(c == 0), stop=(c == 3))
                        nc.vector.tensor_copy(YA[:, ob, qsl], ps)

                # PV of the previous head
                if prev_pts is not None:
                    emit_pv(h - 1, prev_pts, prev_aodt)
                prev_pts, prev_aodt = pts, aodts[p]

                # drain any residual production at pair boundaries
                if par == 1 and fillers is not None:
                    for _ in fillers:
                        pass
                    fillers = None

            emit_pv(HEADS - 1, prev_pts, prev_aodt)
            emit_cleanup(NC - 1, aodts.pop(NC - 1))

        # ---------- proj + bias + out ----------
        with tc.tile_pool(name="ps4", bufs=6, space="PSUM") as ps4_p, \
             tc.tile_pool(name="wpp", bufs=1) as wp2_p:
            YSB = xt_p.tile([128, NC, N], F32, tag="xtslot")
            for ob in range(NC):
                for qh in range(NQH):
                    qsl = slice(qh * 512, (qh + 1) * 512)
                    ps = ps4_p.tile([128, 512], F32, tag="ps4",
                                    name=f"psp{ob}_{qh}")
                    for c in range(4, NC):
                        nc.tensor.matmul(
                            ps, lhsT=WP[:, c, ob * 128:(ob + 1) * 128],
                            rhs=AOD[:, c, qsl],
                            start=(c == 4), stop=(c == NC - 1))
                    nc.vector.scalar_tensor_tensor(
                        YSB[:, ob, qsl], ps, PBIAS[:, ob:ob + 1],
                        YA[:, ob, qsl], ADD, ADD)
                    nc.sync.dma_start(out=y[ob * 128:(ob + 1) * 128, qsl],
                                      in_=YSB[:, ob, qsl])
        aod_p.release()
        xt_p.release()
        cons_p.release()
        vall_p.release()
        kall_p.release()
        qall_p.release()

    nc.compile()
    return nc


def host_prep(x, qkv_w, qkv_b, proj_w, proj_b, rel_pos_h, rel_pos_w):
    """full inputs -> list of 8 per-core in_maps"""
    import ml_dtypes
    x = np.asarray(x, np.float32)
    qkv_w = np.asarray(qkv_w, np.float32)
    qkv_b = np.asarray(qkv_b, np.float32)
    proj_w = np.asarray(proj_w, np.float32)
    proj_b = np.asarray(proj_b, np.float32)
    rel_pos_h = np.asarray(rel_pos_h, np.float32)
    rel_pos_w = np.asarray(rel_pos_w, np.float32)

    wqkvT = np.ascontiguousarray(qkv_w.T).copy()   # (768, 2304)
    wqkvT[:, :DIM] *= SCALE
    qkvb2 = qkv_b.copy()
    qkvb2[:DIM] *= SCALE
    # packed (pair, 768, 384) = q_p | k_p | v_p
    wpack = np.empty((NC, DIM, 384), np.float32)
    for p in range(NC):
        wpack[p, :, 0:128] = wqkvT[:, p * 128:(p + 1) * 128]
        wpack[p, :, 128:256] = wqkvT[:, DIM + p * 128:DIM + (p + 1) * 128]
        wpack[p, :, 256:384] = wqkvT[:, 2 * DIM + p * 128:2 * DIM + (p + 1) * 128]
    wpack = wpack.astype(ml_dtypes.bfloat16)
    wprojT = np.ascontiguousarray(proj_w.T).astype(ml_dtypes.bfloat16)
    projb_f = proj_b + proj_w @ qkvb2[2 * DIM:]

    idx = np.arange(H)
    Rh = rel_pos_h[idx[:, None] - idx[None, :] + (H - 1)]  # (32,32,64) [q,k,c]
    Rw = rel_pos_w[idx[:, None] - idx[None, :] + (W - 1)]
    # block-diagonal rel stationary: rows 0:64 hold (0 | RhT | RwT) for
    # even heads, rows 64:128 hold (RhT | RwT | 0) for odd heads
    bdfull = np.zeros((128, H, 128), np.float32)
    bdfull[0:64, :, 64:96] = Rh.transpose(2, 0, 1) / SCALE
    bdfull[0:64, :, 96:128] = Rw.transpose(2, 0, 1) / SCALE
    bdfull[64:128, :, 0:32] = Rh.transpose(2, 0, 1) / SCALE
    bdfull[64:128, :, 32:64] = Rw.transpose(2, 0, 1) / SCALE
    bdfull = bdfull.astype(ml_dtypes.bfloat16)

    k = np.arange(N)
    kconst = np.zeros((64, N), np.float32)
    kconst[:32] = (k[None, :] // 32 == np.arange(32)[:, None])
    kconst[32:] = (k[None, :] % 32 == np.arange(32)[:, None])
    kconst = kconst.astype(ml_dtypes.bfloat16)

    ident = np.eye(128, dtype=ml_dtypes.bfloat16)

    shared = dict(wpack=wpack, qkvb=qkvb2, wprojT=wprojT, projb=projb_f,
                  bdfull=bdfull, kconst=kconst, ident=ident)
    in_maps = []
    for b in range(B):
        xTb = np.ascontiguousarray(x[b].reshape(N, DIM).T).astype(ml_dtypes.bfloat16)
        in_maps.append(dict(xT=xTb, **shared))
    return in_maps


def get_nc():
    if "nc" not in _CACHE:
        _CACHE["nc"] = build_nc()
    return _CACHE["nc"]


def kernel(**inputs):
    nc = get_nc()
    in_maps = host_prep(**inputs)
    res = bass_utils.run_bass_kernel_spmd(nc, in_maps, core_ids=list(range(NCORES)))
    out = np.stack([np.asarray(r["y"]).T for r in res.results], axis=0)
    return np.ascontiguousarray(out).reshape(B, H, W, DIM).astype(np.float32)
